# revision 1
# baseline (speedup 1.0000x reference)
"""TRN2 Bass kernel for nn_EquivariantConv (GNN message passing).

Strategy (8 NeuronCores):
- Edges assigned to core c by destination node block: col in [c*6250, (c+1)*6250).
- Per core, edges laid out col-node-major with degree padded to multiples of 8,
  packed into 128 SBUF partitions (each dest node's slots live in one
  partition, contiguous along the free dim). This makes:
    * pos[col] a zero-stride broadcast access pattern (free),
    * the segment-sum a dense tensor_reduce (8-slot groups, then per-class
      D/8 runs) - no scatter at all.
- Row records (pos|f_1 packed to 8 f32) are gathered from a DRAM table via
  chunked gpsimd indirect DMA (128 rows per call, one offset per partition).
- Radial MLP computed exactly on TensorE per edge chunk (transpose -> matmul
  x2 -> transpose back); basis embedding built on DVE+ACT.
- Per-node sums are scattered (indirect DMA, 1 call per packed-node column)
  into a per-core local output buffer; host concatenates core slices.

Dummy padding edges point at a zeroed table row -> f_1 = 0 -> f_edge = 0
exactly (all tensor-product terms carry an x factor).
"""

import math
import os
import numpy as np

import concourse.bass as bass
import concourse.bacc as bacc
import concourse.mybir as mybir
from concourse.tile import TileContext
from concourse.bass_utils import run_bass_kernel_spmd

dt = mybir.dt


def _patch_tile_drain():
    """This walrus build rejects drains carrying >1 semaphore wait ("Too many
    sync wait commands"). Split the kernel-tail drain's waits onto separate
    SP drain instructions, one wait each."""
    import concourse.tile as tile_mod
    from concourse.vector_clock import ScopedClock

    if getattr(tile_mod.TileContext, "_drain_patched", False):
        return

    def _drain_and_barrier(self, tick_clock, wait_clock):
        nc = self.nc
        probe = nc.sync.drain()
        wait_clock.add_sem_waits(
            probe.ins, ScopedClock({None: tick_clock.global_clock})
        )
        waits = list(probe.ins.sync_info.on_wait) if probe.ins.sync_info else []
        if len(waits) > 1:
            probe.ins.sync_info.on_wait = waits[:1]
            for w in waits[1:]:
                n2 = nc.sync.drain()
                if n2.ins.sync_info is None:
                    n2.ins.sync_info = mybir.SyncInfo(on_wait=[w], on_update=[])
                else:
                    n2.ins.sync_info.on_wait = [w]
        nc.all_engine_barrier()
        popped = nc._tile_sem_poison_stack.pop()
        assert popped is self._sem_poison
        nc.clear_and_free_semaphores(list(self.sems.allocated().values()))
        nc.all_engine_barrier()

    tile_mod.TileContext._drain_and_barrier = _drain_and_barrier
    tile_mod.TileContext._drain_patched = True


def _install_ntff_shim():
    """Optional: enable NTFF profiling under axon (antenv.axon_hooks shim)."""
    import contextlib
    import ctypes
    import sys
    import types

    if "antenv.axon_hooks" in sys.modules:
        return
    so_path = "/opt/axon/libaxon_pjrt.so"
    if not os.path.exists(so_path):
        return
    try:
        lib = ctypes.CDLL(so_path)
        if not hasattr(lib, "axon_start_nrt_profile"):
            return
        lib.axon_start_nrt_profile.argtypes = [
            ctypes.POINTER(ctypes.c_int64), ctypes.c_size_t]
        lib.axon_start_nrt_profile.restype = ctypes.c_int64
        lib.axon_stop_nrt_profile.argtypes = [ctypes.c_char_p]
        lib.axon_stop_nrt_profile.restype = ctypes.c_int64

        @contextlib.contextmanager
        def _profile(output_dir, device_ids):
            import jax
            jax.devices()
            if device_ids:
                ids = (ctypes.c_int64 * len(device_ids))(*device_ids)
                rc = lib.axon_start_nrt_profile(ids, len(device_ids))
            else:
                rc = lib.axon_start_nrt_profile(None, 0)
            if rc != 0:
                raise RuntimeError(f"axon_start_nrt_profile rc={rc}")
            try:
                yield
            finally:
                lib.axon_stop_nrt_profile(output_dir.encode())

        mod = types.ModuleType("antenv.axon_hooks")
        mod.get_axon_ntff_profile_hook = lambda: _profile
        mod.set_axon_ntff_profile_hook = lambda h: None
        import antenv
        antenv.axon_hooks = mod
        sys.modules["antenv.axon_hooks"] = mod
    except Exception:
        pass


_patch_tile_drain()

LAST_EXEC_NS = None
Alu = mybir.AluOpType
Act = mybir.ActivationFunctionType

N_NODES = 50000
N_EDGES = 1600000
NUM_BASIS = 10
HIDDEN = 64
MAX_RADIUS = 3.0
N_CORES = 8
NPC = N_NODES // N_CORES  # dest nodes per core
P = 128

# table rows: N_NODES real + row N_NODES..N_NODES+7 zeroed (dummy target)
TBL_ROWS = N_NODES + 8
REC = 8  # packed record: [pos_x, pos_y, pos_z, f0, f1, f2, f3, 0]

FC = 128          # chunk width (free-dim columns); 128*FC slots per chunk
MM_FREE = 512     # matmul free-dim tile


def _pack_edges(row, col_local, rng):
    """Pack one core's edges into the partitioned, class-structured layout.

    Returns (classes, part_nodes, part_rows) where
      classes: list of (D_k, n_k) with uniform per-partition node counts
      part_nodes: [P][sum n_k] local dest-node id per packed node (or -1 dummy)
      part_rows:  [P][F] global row id per slot (or TBL dummy row)
    Layout per partition: for class k in order, n_k nodes, each D_k slots.
    """
    deg = np.bincount(col_local, minlength=NPC)
    order = np.argsort(col_local, kind="stable")
    row_sorted = row[order]
    starts = np.zeros(NPC + 1, np.int64)
    np.cumsum(deg, out=starts[1:])

    # class of node = ceil(deg/4)*4 (deg 0 nodes are skipped entirely)
    nz = np.nonzero(deg)[0]
    dcls = ((deg[nz] + 3) // 4) * 4
    classes = sorted(set(dcls.tolist()))
    per_class_nodes = {c: nz[dcls == c] for c in classes}
    return deg, starts, row_sorted, classes, per_class_nodes


def _build_layout(edge_index):
    """Host-side index work: per-core slot layout. Values untouched.

    Cross-core class balancing: per-partition class counts n_k are chosen
    globally from suffix maxima of per-core padded-degree histograms; cores
    short on class-k nodes promote lower-degree nodes into the larger class
    (extra slots become dummy edges). This removes most cross-core padding.
    """
    row = edge_index[0].astype(np.int64)
    col = edge_index[1].astype(np.int64)
    core = col // NPC

    per_core = []
    for c in range(N_CORES):
        m = core == c
        row_c = row[m]
        col_c = col[m] - c * NPC
        deg = np.bincount(col_c, minlength=NPC)
        order = np.argsort(col_c, kind="stable")
        row_sorted = row_c[order]
        starts = np.zeros(NPC + 1, np.int64)
        np.cumsum(deg, out=starts[1:])
        nz = np.nonzero(deg)[0]
        pdeg = ((deg[nz] + 3) // 4) * 4
        per_core.append((deg, starts, row_sorted, nz, pdeg))

    # global class sizing: S_k = max over cores of #nodes with pdeg >= k
    all_k = sorted({int(v) for (_, _, _, _, pdeg) in per_core for v in pdeg},
                   reverse=True)
    n_k = {}
    cum = 0  # per-partition slots already committed to classes > k
    for k in all_k:
        s_k = max(int((pd >= k).sum()) for (_, _, _, _, pd) in per_core)
        need = max((s_k + P - 1) // P, cum)
        n_k[k] = need - cum
        cum = need
    class_list = [(k, n_k[k]) for k in all_k if n_k[k] > 0]
    class_list = class_list[::-1]  # ascending k, as the device program expects

    NN = sum(nk for (_, nk) in class_list)
    F = sum(nk * k for (k, nk) in class_list)
    F_pad = (F + FC - 1) // FC * FC

    row_slots = np.full((N_CORES, P, F_pad), N_NODES, np.int32)
    node_gid = np.full((N_CORES, P, NN), N_NODES, np.int32)
    node_lid = np.full((N_CORES, P, NN), NPC + 6, np.int32)

    # per-class slot bases (ascending class order = device layout order)
    foffs = {}
    noffs = {}
    fo = 0
    no = 0
    for (k, nk) in class_list:
        foffs[k] = fo
        noffs[k] = no
        fo += nk * k
        no += nk

    desc = [k for (k, _) in class_list][::-1]
    for c in range(N_CORES):
        deg, starts, row_sorted, nz, pdeg = per_core[c]
        # nodes sorted by padded degree desc; assign to class slots desc
        order = np.argsort(-pdeg, kind="stable")
        nodes_desc = nz[order]
        pos_in_class = 0
        ki = 0
        for n in nodes_desc:
            while pos_in_class >= n_k[desc[ki]] * P:
                ki += 1
                pos_in_class = 0
            k = desc[ki]
            j = pos_in_class  # global slot index within class k
            p = j % P
            jj = j // P
            d = deg[n]
            f0 = foffs[k] + jj * k
            row_slots[c, p, f0:f0 + d] = row_sorted[starts[n]:starts[n + 1]]
            node_gid[c, p, noffs[k] + jj] = c * NPC + n
            node_lid[c, p, noffs[k] + jj] = n
            pos_in_class += 1
    return class_list, NN, F_pad, row_slots, node_gid, node_lid


def _build_program(class_list, NN, F):
    """Emit the Bass program (same for all cores; per-core data differs)."""
    nc = bacc.Bacc(None)
    f_1 = nc.declare_dram_parameter("f_1", [N_NODES, 4], dt.float32, isOutput=False)
    pos = nc.declare_dram_parameter("pos", [N_NODES, 3], dt.float32, isOutput=False)
    W1 = nc.declare_dram_parameter("W1", [NUM_BASIS, HIDDEN], dt.float32, isOutput=False)
    W2 = nc.declare_dram_parameter("W2", [HIDDEN, 5], dt.float32, isOutput=False)
    rowidx = nc.declare_dram_parameter("rowidx", [P, F], dt.int32, isOutput=False)
    ngid = nc.declare_dram_parameter("ngid", [P, NN], dt.int32, isOutput=False)
    nlid = nc.declare_dram_parameter("nlid", [P, NN], dt.int32, isOutput=False)
    yout = nc.declare_dram_parameter("yout", [NPC, 4], dt.float32, isOutput=True)

    rec = nc.dram_tensor("rec", [TBL_ROWS, REC], dt.float32)

    n_chunks = F // FC
    C_EMB = 1.14136 * float(np.e) ** 2
    # fold: relu *sqrt(2); W1 /sqrt(10); emb *C ; W2 /sqrt(64); /sqrt(32) nbrs
    w1_scale = C_EMB / math.sqrt(NUM_BASIS)
    w2_common = math.sqrt(2.0) / math.sqrt(HIDDEN) / math.sqrt(32.0)
    col_scales = [
        math.sqrt(0.5) * w2_common,            # a0 (w0 * x0)
        1.0 * w2_common,                       # a1 (w1 * x0 * u)
        (1.0 / math.sqrt(3.0)) * w2_common,    # a2 (w2 * xv)
        math.sqrt(0.5) * w2_common,            # a3 (w3 * dot)
        (1.0 / math.sqrt(2.0)) * w2_common,    # a4 (w4 * cross)
    ]

    with TileContext(nc) as tc:
        with (
            tc.tile_pool(name="persist", bufs=1) as pp,
            tc.tile_pool(name="chunk", bufs=2) as cp,
            tc.tile_pool(name="recp", bufs=4) as rp,
            tc.tile_pool(name="psum", bufs=2, space="PSUM") as psp,
            tc.tile_pool(name="mmp", bufs=2, space="PSUM") as mmp,
        ):
            # ---- stage 0: build packed node record table in DRAM ----
            # zero the dummy tail rows + assemble [pos | f1 | 0]
            ztile = pp.tile([P, REC], dt.float32)
            nc.vector.memset(ztile[:], 0.0)
            nc.sync.dma_start(out=rec[N_NODES:TBL_ROWS, :], in_=ztile[:8, :])
            # pos -> rec[:, 0:3] ; f_1 -> rec[:, 3:7]; rec[:,7] zero
            # bounce via SBUF in 2048-row blocks ([128 partitions x 16 rows])
            G0 = 64
            BLK = P * G0
            for b in range((N_NODES + BLK - 1) // BLK):
                r0 = b * BLK
                r1 = min(r0 + BLK, N_NODES)
                n = r1 - r0
                gfull = n // G0  # partitions fully covered (n multiple of G0?)
                t = cp.tile([P, G0, REC], dt.float32, tag="recb", name="recb")
                if n == BLK:
                    nc.sync.dma_start(out=t[:, :, 0:3], in_=pos[r0:r1, :])
                    nc.sync.dma_start(out=t[:, :, 3:7], in_=f_1[r0:r1, :])
                    nc.sync.dma_start(out=rec[r0:r1, :], in_=t[:])
                else:
                    # tail: per-128-row sub-blocks
                    for bb in range((n + P - 1) // P):
                        s0 = r0 + bb * P
                        s1 = min(s0 + P, N_NODES)
                        m = s1 - s0
                        tt = cp.tile([P, REC], dt.float32, tag="recbt", name="recbt")
                        nc.vector.memset(tt[:], 0.0)
                        nc.sync.dma_start(out=tt[:m, 0:3], in_=pos[s0:s1, :])
                        nc.sync.dma_start(out=tt[:m, 3:7], in_=f_1[s0:s1, :])
                        nc.sync.dma_start(out=rec[s0:s1, :], in_=tt[:m, :])

            # ---- stage 0b: weights prep ----
            # block-diagonal weights: 2 edge-columns per matmul set
            w1blk = pp.tile([2 * NUM_BASIS, P], dt.float32)
            nc.vector.memset(w1blk[:], 0.0)
            nc.sync.dma_start(out=w1blk[0:NUM_BASIS, 0:HIDDEN], in_=W1[:])
            nc.sync.dma_start(out=w1blk[NUM_BASIS:2 * NUM_BASIS, HIDDEN:2 * HIDDEN],
                              in_=W1[:])
            nc.vector.tensor_scalar_mul(w1blk[:], w1blk[:], w1_scale)
            w2blk = pp.tile([P, 2 * 5], dt.float32)
            nc.vector.memset(w2blk[:], 0.0)
            nc.sync.dma_start(out=w2blk[0:HIDDEN, 0:5], in_=W2[:])
            nc.sync.dma_start(out=w2blk[HIDDEN:2 * HIDDEN, 5:10], in_=W2[:])
            for j, s in enumerate(col_scales):
                nc.vector.tensor_scalar_mul(w2blk[:, j:j + 1], w2blk[:, j:j + 1], s)
                nc.vector.tensor_scalar_mul(w2blk[:, 5 + j:6 + j], w2blk[:, 5 + j:6 + j], s)
            ident = pp.tile([P, P], dt.float32)
            bconst = pp.tile([P, NUM_BASIS], dt.float32)
            for k in range(NUM_BASIS):
                nc.vector.memset(bconst[:, k:k + 1], -(k + 1.0))
            from concourse.masks import make_identity
            make_identity(nc, ident[:])

            # ---- stage 1: load index arrays ----
            ridx = pp.tile([P, F], dt.int32)
            nc.sync.dma_start(out=ridx[:], in_=rowidx[:])
            gidt = pp.tile([P, NN], dt.int32)
            nc.sync.dma_start(out=gidt[:], in_=ngid[:])
            lidt = pp.tile([P, NN], dt.int32)
            nc.sync.dma_start(out=lidt[:], in_=nlid[:])

            # ---- stage 2: gather packed col-node records [P, NN, 8] ----
            # (full records: the dynamic-AP coefficient is derived from the
            # indirect tensor's shape, so a column slice would mis-stride)
            pcol = pp.tile([P, NN, REC], dt.float32, name="pcol")
            for j in range(NN):
                nc.gpsimd.indirect_dma_start(
                    out=pcol[:, j, :],
                    out_offset=None,
                    in_=rec[:],
                    in_offset=bass.IndirectOffsetOnAxis(ap=gidt[:, j:j + 1], axis=0),
                )

            # expand pos[col] to slot-aligned slabs [P, F] per component
            pcx = pp.tile([P, F], dt.float32, tag="pcx", name="pcx")
            pcy = pp.tile([P, F], dt.float32, tag="pcy", name="pcy")
            pcz = pp.tile([P, F], dt.float32, tag="pcz", name="pcz")
            foff = 0
            noff = 0
            for (k, nk) in class_list:
                for comp, dst in ((0, pcx), (1, pcy), (2, pcz)):
                    src = pcol[:, noff:noff + nk, comp:comp + 1]  # [P, nk, 1]
                    nc.vector.tensor_copy(
                        out=dst[:, foff:foff + nk * k].rearrange(
                            "p (n d) -> p n d", d=k),
                        in_=src.to_broadcast([P, nk, k]),
                    )
                foff += nk * k
                noff += nk

            # persistent 4-group sums [P, F/4] per component
            F8 = F // 4
            g8 = [pp.tile([P, F8], dt.float32, tag=f"g8_{i}", name=f"g8_{i}") for i in range(4)]

            # ---- stage 3: per-chunk pipeline ----
            for ch in range(n_chunks):
                c0 = ch * FC
                recc = rp.tile([P, FC, REC], dt.float32, tag="recc", name="recc")
                for t in range(FC):
                    nc.gpsimd.indirect_dma_start(
                        out=recc[:, t, :],
                        out_offset=None,
                        in_=rec[:],
                        in_offset=bass.IndirectOffsetOnAxis(
                            ap=ridx[:, c0 + t:c0 + t + 1], axis=0),
                    )

                prx = recc[:, :, 0]
                pry = recc[:, :, 1]
                prz = recc[:, :, 2]
                x0 = recc[:, :, 3]
                x1 = recc[:, :, 4]
                x2 = recc[:, :, 5]
                x3 = recc[:, :, 6]

                def T(tag):
                    return cp.tile([P, FC], dt.float32, tag=tag, name=tag)

                evx, evy, evz = T("evx"), T("evy"), T("evz")
                nc.vector.tensor_tensor(out=evx[:], in0=prx, in1=pcx[:, c0:c0 + FC], op=Alu.subtract)
                nc.vector.tensor_tensor(out=evy[:], in0=pry, in1=pcy[:, c0:c0 + FC], op=Alu.subtract)
                nc.vector.tensor_tensor(out=evz[:], in0=prz, in1=pcz[:, c0:c0 + FC], op=Alu.subtract)
                r2 = T("r2")
                tmp = T("tmp")
                nc.vector.tensor_tensor(out=r2[:], in0=evx[:], in1=evx[:], op=Alu.mult)
                nc.vector.tensor_tensor(out=tmp[:], in0=evy[:], in1=evy[:], op=Alu.mult)
                nc.vector.tensor_tensor(out=r2[:], in0=r2[:], in1=tmp[:], op=Alu.add)
                nc.vector.tensor_tensor(out=tmp[:], in0=evz[:], in1=evz[:], op=Alu.mult)
                nc.vector.tensor_tensor(out=r2[:], in0=r2[:], in1=tmp[:], op=Alu.add)
                nc.vector.tensor_scalar_max(r2[:], r2[:], 1e-12)
                r = T("r")
                nc.scalar.sqrt(out=r[:], in_=r2[:])
                rinv = T("rinv")
                nc.vector.reciprocal(out=rinv[:], in_=r[:])
                ux, uy, uz = T("ux"), T("uy"), T("uz")
                nc.vector.tensor_tensor(out=ux[:], in0=evx[:], in1=rinv[:], op=Alu.mult)
                nc.vector.tensor_tensor(out=uy[:], in0=evy[:], in1=rinv[:], op=Alu.mult)
                nc.vector.tensor_tensor(out=uz[:], in0=evz[:], in1=rinv[:], op=Alu.mult)
                # e3nn (y,z,x) permuted unit vector
                up1, up2, up3 = uy, uz, ux

                # ---- embedding [P, FC, 10] ----
                embs = cp.tile([P, FC, NUM_BASIS], dt.float32, tag="embs", name="embs")
                uk = T("uk")
                den = T("den")
                vv = T("vv")
                ee = T("ee")
                mk = T("mk")
                for k in range(NUM_BASIS):
                    # d = 4r - (k+1); uk = d^2
                    # d_k = (r - v_k)/step with v_k = 3(k+1)/11, step = 3/11
                    nc.scalar.activation(out=uk[:], in_=r[:], func=Act.Square,
                                         bias=bconst[:, k:k + 1], scale=11.0 / 3.0)
                    nc.vector.tensor_scalar_add(den[:], uk[:], -1.0)
                    nc.vector.reciprocal(out=vv[:], in_=den[:])
                    # clamp: outside the bin u>1 makes v large positive and
                    # exp overflows to inf; inf * 0-mask = NaN. Inside, v<0.
                    nc.vector.tensor_scalar_min(vv[:], vv[:], 0.0)
                    nc.scalar.activation(out=ee[:], in_=vv[:], func=Act.Exp,
                                         bias=0.0, scale=2.0)
                    nc.vector.tensor_scalar(out=mk[:], in0=uk[:], scalar1=1.0,
                                            scalar2=None, op0=Alu.is_lt)
                    nc.vector.tensor_tensor(out=embs[:, :, k], in0=ee[:],
                                            in1=mk[:], op=Alu.mult)

                # ---- radial MLP on PE (block-diag, 8 cols / 1024 edges) ----
                wsl = cp.tile([P, FC, 5], dt.float32, tag="wsl", name="wsl")
                for m0 in range(0, FC, 8):
                    embT = psp.tile([2 * NUM_BASIS, 512], dt.float32,
                                    space="PSUM", tag="embT", name="embT")
                    for q in range(4):
                        nc.tensor.transpose(
                            out=embT[:, q * P:(q + 1) * P],
                            in_=embs[:, m0 + 2 * q:m0 + 2 * q + 2, :],
                            identity=ident[:])
                    embTs = cp.tile([2 * NUM_BASIS, 512], dt.float32,
                                    tag="embTs", name="embTs")
                    nc.scalar.copy(out=embTs[:], in_=embT[:])
                    hpsum = mmp.tile([P, 512], dt.float32,
                                     space="PSUM", tag="hpsum", name="hpsum")
                    nc.tensor.matmul(out=hpsum[:], lhsT=w1blk[:], rhs=embTs[:],
                                     start=True, stop=True)
                    hs = cp.tile([P, 512], dt.float32, tag="hs", name="hs")
                    nc.scalar.activation(out=hs[:], in_=hpsum[:], func=Act.Relu)
                    wpsum = mmp.tile([2 * 5, 512], dt.float32, space="PSUM",
                                     tag="wpsum", name="wpsum")
                    nc.tensor.matmul(out=wpsum[:], lhsT=w2blk[:], rhs=hs[:],
                                     start=True, stop=True)
                    ws = cp.tile([2 * 5, 512], dt.float32, tag="ws", name="ws")
                    nc.scalar.copy(out=ws[:], in_=wpsum[:])
                    wT = psp.tile([P, 8 * 5], dt.float32, space="PSUM",
                                  tag="wT", name="wT")
                    for q in range(4):
                        nc.tensor.transpose(
                            out=wT[:, q * 10:(q + 1) * 10],
                            in_=ws[:, q * P:(q + 1) * P],
                            identity=ident[:10, :10])
                    nc.vector.tensor_copy(
                        out=wsl[:, m0:m0 + 8, :], in_=wT[:])

                w0 = wsl[:, :, 0]
                w1_ = wsl[:, :, 1]
                w2_ = wsl[:, :, 2]
                w3 = wsl[:, :, 3]
                w4 = wsl[:, :, 4]

                # ---- tensor product ----
                dv = T("dv")
                nc.vector.tensor_tensor(out=dv[:], in0=x1, in1=up1[:], op=Alu.mult)
                nc.vector.tensor_tensor(out=tmp[:], in0=x2, in1=up2[:], op=Alu.mult)
                nc.vector.tensor_tensor(out=dv[:], in0=dv[:], in1=tmp[:], op=Alu.add)
                nc.vector.tensor_tensor(out=tmp[:], in0=x3, in1=up3[:], op=Alu.mult)
                nc.vector.tensor_tensor(out=dv[:], in0=dv[:], in1=tmp[:], op=Alu.add)

                o0 = T("o0")
                nc.vector.tensor_tensor(out=o0[:], in0=w0, in1=x0, op=Alu.mult)
                nc.vector.tensor_tensor(out=tmp[:], in0=w3, in1=dv[:], op=Alu.mult)
                nc.vector.tensor_tensor(out=o0[:], in0=o0[:], in1=tmp[:], op=Alu.add)

                t1 = T("t1")
                nc.vector.tensor_tensor(out=t1[:], in0=w1_, in1=x0, op=Alu.mult)

                # cross(xv, up)
                cr1, cr2, cr3 = T("cr1"), T("cr2"), T("cr3")
                nc.vector.tensor_tensor(out=cr1[:], in0=x2, in1=up3[:], op=Alu.mult)
                nc.vector.tensor_tensor(out=tmp[:], in0=x3, in1=up2[:], op=Alu.mult)
                nc.vector.tensor_tensor(out=cr1[:], in0=cr1[:], in1=tmp[:], op=Alu.subtract)
                nc.vector.tensor_tensor(out=cr2[:], in0=x3, in1=up1[:], op=Alu.mult)
                nc.vector.tensor_tensor(out=tmp[:], in0=x1, in1=up3[:], op=Alu.mult)
                nc.vector.tensor_tensor(out=cr2[:], in0=cr2[:], in1=tmp[:], op=Alu.subtract)
                nc.vector.tensor_tensor(out=cr3[:], in0=x1, in1=up2[:], op=Alu.mult)
                nc.vector.tensor_tensor(out=tmp[:], in0=x2, in1=up1[:], op=Alu.mult)
                nc.vector.tensor_tensor(out=cr3[:], in0=cr3[:], in1=tmp[:], op=Alu.subtract)

                o1, o2, o3 = T("o1"), T("o2"), T("o3")
                for oo, upc, xc, crc in ((o1, up1, x1, cr1), (o2, up2, x2, cr2),
                                         (o3, up3, x3, cr3)):
                    nc.vector.tensor_tensor(out=oo[:], in0=t1[:], in1=upc[:], op=Alu.mult)
                    nc.vector.tensor_tensor(out=tmp[:], in0=w2_, in1=xc, op=Alu.mult)
                    nc.vector.tensor_tensor(out=oo[:], in0=oo[:], in1=tmp[:], op=Alu.add)
                    nc.vector.tensor_tensor(out=tmp[:], in0=w4, in1=crc[:], op=Alu.mult)
                    nc.vector.tensor_tensor(out=oo[:], in0=oo[:], in1=tmp[:], op=Alu.add)

                # ---- 8-group partial reduction into persistent slabs ----
                for i, oo in enumerate((o0, o1, o2, o3)):
                    nc.vector.tensor_reduce(
                        out=g8[i][:, ch * FC // 4:(ch + 1) * FC // 4],
                        in_=oo[:].rearrange("p (g e) -> p g e", e=4),
                        op=Alu.add,
                        axis=mybir.AxisListType.X,
                    )

            # ---- stage 4: per-class final reduction [P, NN] x4 ----
            nsum = pp.tile([P, NN, 4], dt.float32, name="nsum")
            foff8 = 0
            noff = 0
            for (k, nk) in class_list:
                k8 = k // 4
                for i in range(4):
                    nc.vector.tensor_reduce(
                        out=nsum[:, noff:noff + nk, i],
                        in_=g8[i][:, foff8:foff8 + nk * k8].rearrange(
                            "p (n g) -> p n g", g=k8),
                        op=Alu.add,
                        axis=mybir.AxisListType.X,
                    )
                foff8 += nk * k8
                noff += nk

            # ---- stage 5: scatter per-node sums directly into yout ----
            # dummy nodes carry lid > NPC-1; bounds_check silently drops them.
            # PJRT hands the kernel pre-zeroed output buffers, so deg-0 nodes
            # (never packed/written) correctly stay 0.
            for j in range(NN):
                nc.gpsimd.indirect_dma_start(
                    out=yout[:],
                    out_offset=bass.IndirectOffsetOnAxis(ap=lidt[:, j:j + 1], axis=0),
                    in_=nsum[:, j, :],
                    in_offset=None,
                    bounds_check=NPC - 1,
                    oob_is_err=False,
                )

    nc.finalize()
    return nc


def kernel(f_1, pos, W1, W2, edge_index):
    f_1 = np.ascontiguousarray(f_1, np.float32)
    pos = np.ascontiguousarray(pos, np.float32)
    W1 = np.ascontiguousarray(W1, np.float32)
    W2 = np.ascontiguousarray(W2, np.float32)
    ei = np.asarray(edge_index).astype(np.int64)

    class_list, NN, F, row_slots, node_gid, node_lid = _build_layout(ei)
    nc = _build_program(class_list, NN, F)

    in_maps = []
    for c in range(N_CORES):
        in_maps.append({
            "f_1": f_1, "pos": pos, "W1": W1, "W2": W2,
            "rowidx": row_slots[c], "ngid": node_gid[c], "nlid": node_lid[c],
        })
    trace = os.environ.get("KERNEL_TRACE", "0") == "1"
    if trace:
        _install_ntff_shim()
    res = run_bass_kernel_spmd(nc, in_maps, list(range(N_CORES)), trace=trace)
    global LAST_EXEC_NS
    LAST_EXEC_NS = res.exec_time_ns
    out = np.concatenate([res.results[c]["yout"] for c in range(N_CORES)], axis=0)
    return out.astype(np.float32)


if __name__ == "__main__":
    import reference
    inputs = {k: np.asarray(v) for k, v in reference.setup_inputs().items()}
    out = kernel(**inputs)
    print("kernel out", out.shape, out.dtype)



# revision 21
# speedup vs baseline: 1.7470x; 1.7470x over previous
"""TRN2 Bass kernel for nn_EquivariantConv (GNN message passing).

Strategy (8 NeuronCores):
- Edges assigned to core c by destination node block: col in [c*6250, (c+1)*6250).
- Per core, edges laid out col-node-major with degree padded to multiples of 4,
  packed into 128 SBUF partitions (each dest node's slots live in one
  partition, contiguous along the free dim). This makes:
    * pos[col] a zero-stride broadcast access pattern (free),
    * the segment-sum a dense tensor_reduce - no scatter at all.
- Source-row records (pos|f_1 packed to 8 f32) are gathered from a DRAM table
  via gpsimd dma_gather (SWDGE): the table packs TWO records per 256B row
  (gather element granularity), idx = row//2 fits int16, and a 3-op DVE
  parity select picks the right half. 1024 indices per call (desc-ring cap).
- Radial MLP computed exactly on TensorE per edge chunk (transpose -> matmul
  x2 -> transpose back); basis embedding built on DVE+ACT (reciprocal on ACT).
- Per-node sums scattered with dma_scatter_add into a 256B-strided output
  buffer (trash rows absorb dummy slots); host slices the real rows.

Dummy padding edges point at a zeroed table row -> f_1 = 0 -> f_edge = 0
exactly (all tensor-product terms carry an x factor).
"""

import math
import os
import numpy as np

import concourse.bass as bass
import concourse.bacc as bacc
import concourse.mybir as mybir
from concourse.tile import TileContext
from concourse.bass_utils import run_bass_kernel_spmd

dt = mybir.dt


def _patch_tile_drain():
    """This walrus build rejects drains carrying >1 semaphore wait ("Too many
    sync wait commands"). Split the kernel-tail drain's waits onto separate
    SP drain instructions, one wait each."""
    import concourse.tile as tile_mod
    from concourse.vector_clock import ScopedClock

    if getattr(tile_mod.TileContext, "_drain_patched", False):
        return

    def _drain_and_barrier(self, tick_clock, wait_clock):
        nc = self.nc
        probe = nc.sync.drain()
        wait_clock.add_sem_waits(
            probe.ins, ScopedClock({None: tick_clock.global_clock})
        )
        waits = list(probe.ins.sync_info.on_wait) if probe.ins.sync_info else []
        if len(waits) > 1:
            probe.ins.sync_info.on_wait = waits[:1]
            for w in waits[1:]:
                n2 = nc.sync.drain()
                if n2.ins.sync_info is None:
                    n2.ins.sync_info = mybir.SyncInfo(on_wait=[w], on_update=[])
                else:
                    n2.ins.sync_info.on_wait = [w]
        nc.all_engine_barrier()
        popped = nc._tile_sem_poison_stack.pop()
        assert popped is self._sem_poison
        nc.clear_and_free_semaphores(list(self.sems.allocated().values()))
        nc.all_engine_barrier()

    tile_mod.TileContext._drain_and_barrier = _drain_and_barrier
    tile_mod.TileContext._drain_patched = True


def _install_ntff_shim():
    """Optional: enable NTFF profiling under axon (antenv.axon_hooks shim)."""
    import contextlib
    import ctypes
    import sys
    import types

    if "antenv.axon_hooks" in sys.modules:
        return
    so_path = "/opt/axon/libaxon_pjrt.so"
    if not os.path.exists(so_path):
        return
    try:
        lib = ctypes.CDLL(so_path)
        if not hasattr(lib, "axon_start_nrt_profile"):
            return
        lib.axon_start_nrt_profile.argtypes = [
            ctypes.POINTER(ctypes.c_int64), ctypes.c_size_t]
        lib.axon_start_nrt_profile.restype = ctypes.c_int64
        lib.axon_stop_nrt_profile.argtypes = [ctypes.c_char_p]
        lib.axon_stop_nrt_profile.restype = ctypes.c_int64

        @contextlib.contextmanager
        def _profile(output_dir, device_ids):
            import jax
            jax.devices()
            if device_ids:
                ids = (ctypes.c_int64 * len(device_ids))(*device_ids)
                rc = lib.axon_start_nrt_profile(ids, len(device_ids))
            else:
                rc = lib.axon_start_nrt_profile(None, 0)
            if rc != 0:
                raise RuntimeError(f"axon_start_nrt_profile rc={rc}")
            try:
                yield
            finally:
                lib.axon_stop_nrt_profile(output_dir.encode())

        mod = types.ModuleType("antenv.axon_hooks")
        mod.get_axon_ntff_profile_hook = lambda: _profile
        mod.set_axon_ntff_profile_hook = lambda h: None
        import antenv
        antenv.axon_hooks = mod
        sys.modules["antenv.axon_hooks"] = mod
    except Exception:
        pass


_patch_tile_drain()

LAST_EXEC_NS = None
Alu = mybir.AluOpType
Act = mybir.ActivationFunctionType

N_NODES = 50000
N_EDGES = 1600000
NUM_BASIS = 10
HIDDEN = 64
MAX_RADIUS = 3.0
N_CORES = 8
NPC = N_NODES // N_CORES  # dest nodes per core
P = 128

# table rows: N_NODES real + rows N_NODES..N_NODES+47 zeroed (dummy target)
TBL_ROWS = N_NODES + 48
REC = 8            # packed record: [pos_x, pos_y, pos_z, f0, f1, f2, f3, pad]
PAIRS = TBL_ROWS // 2          # 25024 gather elements of 2 records / 256B
GELEM = 64                     # gather element: 64 f32 = 256B
DUMMY_ROW = N_NODES            # zeroed record row

FC = 128          # chunk width (free-dim columns); 128*FC slots per chunk
NPG = 1024        # indices per dma_gather/dma_scatter_add call (ring cap)
NQUEUES = 4       # SWDGE queues; round-robin hides per-queue DMA round-trip
YROWS = NPC + 22  # output rows plus trash rows for dummy slots (lid NPC+6)


def _dma_gather64(nc, out_ap, in_ap, idxs_ap, num_idxs, queue_num):
    """dma_gather of 64B elements (16 f32) on a 256B-strided table.

    bass's dma_gather asserts elem_size_bytes % 256 == 0, but that is a
    transpose-mode restriction; the non-transpose ucode only needs the row
    STRIDE to be a 256B multiple (stride_bytes_256 descriptor field).
    Verified on hardware. in_ap must be tbl[:, 0:16] of a [N, 64] f32 tensor.
    """
    g = nc.gpsimd
    stride_bytes = GELEM * 4
    _in_ap = g.lower_ap_dma(in_ap, for_custom_bir_dma=True)
    _idxs_ap = g.lower_ap(idxs_ap)
    _out_ap = g.lower_ap(out_ap)
    return g.add_instruction(
        mybir.InstDMAGatherAnt(
            name=nc.get_next_instruction_name(),
            ins=[*_in_ap, _idxs_ap, g.lower_val_access(g.to_reg(num_idxs))],
            outs=[_out_ap],
            transpose=False,
            num_idxs=num_idxs,
            elem_size=16,
            stride_bytes_256=stride_bytes // 256,
            gen_mode=0,
            single_packet=True,
            queue_num=queue_num,
            sbuf_tokens_per_rank=0,
            sbuf_free_dim_per_rank=0,
            sbuf_free_dim_pad_per_rank=0,
            sbuf_byte_offset=0,
        ))


def _wrap16(lin):
    """[N] int array -> [128, N/16] int16 wrapped-16, replicated across the 8
    gpsimd cores (partition p holds lin[j*16 + p%16] at free pos j)."""
    n = lin.shape[0]
    assert n % 16 == 0
    w16 = lin.reshape(n // 16, 16).T.astype(np.int16)  # [16, n/16]
    return np.tile(w16, (8, 1))                        # [128, n/16]


def _build_layout(edge_index):
    """Host-side index work: per-core slot layout. Values untouched.

    Cross-core class balancing: per-partition class counts n_k are chosen
    globally from suffix maxima of per-core padded-degree histograms; cores
    short on class-k nodes promote lower-degree nodes into the larger class
    (extra slots become dummy edges). This removes most cross-core padding.
    """
    row = edge_index[0].astype(np.int64)
    col = edge_index[1].astype(np.int64)
    core = col // NPC

    per_core = []
    for c in range(N_CORES):
        m = core == c
        row_c = row[m]
        col_c = col[m] - c * NPC
        deg = np.bincount(col_c, minlength=NPC)
        order = np.argsort(col_c, kind="stable")
        row_sorted = row_c[order]
        starts = np.zeros(NPC + 1, np.int64)
        np.cumsum(deg, out=starts[1:])
        nz = np.nonzero(deg)[0]
        pdeg = ((deg[nz] + 3) // 4) * 4
        per_core.append((deg, starts, row_sorted, nz, pdeg))

    # global class sizing: S_k = max over cores of #nodes with pdeg >= k
    all_k = sorted({int(v) for (_, _, _, _, pdeg) in per_core for v in pdeg},
                   reverse=True)
    n_k = {}
    cum = 0  # per-partition slots already committed to classes > k
    for k in all_k:
        s_k = max(int((pd >= k).sum()) for (_, _, _, _, pd) in per_core)
        need = max((s_k + P - 1) // P, cum)
        n_k[k] = need - cum
        cum = need
    class_list = [(k, n_k[k]) for k in all_k if n_k[k] > 0]
    class_list = class_list[::-1]  # ascending k, as the device program expects

    NN = sum(nk for (_, nk) in class_list)
    F = sum(nk * k for (k, nk) in class_list)
    F_pad = (F + FC - 1) // FC * FC

    row_slots = np.full((N_CORES, P, F_pad), DUMMY_ROW, np.int32)
    node_gid = np.full((N_CORES, P, NN), DUMMY_ROW, np.int32)
    node_lid = np.full((N_CORES, P, NN), NPC + 6, np.int32)

    # per-class slot bases (ascending class order = device layout order)
    foffs = {}
    noffs = {}
    fo = 0
    no = 0
    for (k, nk) in class_list:
        foffs[k] = fo
        noffs[k] = no
        fo += nk * k
        no += nk

    desc = [k for (k, _) in class_list][::-1]
    for c in range(N_CORES):
        deg, starts, row_sorted, nz, pdeg = per_core[c]
        # nodes sorted by padded degree desc; assign to class slots desc
        order = np.argsort(-pdeg, kind="stable")
        nodes_desc = nz[order]
        pos_in_class = 0
        ki = 0
        for n in nodes_desc:
            while pos_in_class >= n_k[desc[ki]] * P:
                ki += 1
                pos_in_class = 0
            k = desc[ki]
            j = pos_in_class  # global slot index within class k
            p = j % P
            jj = j // P
            d = deg[n]
            f0 = foffs[k] + jj * k
            row_slots[c, p, f0:f0 + d] = row_sorted[starts[n]:starts[n + 1]]
            node_gid[c, p, noffs[k] + jj] = c * NPC + n
            node_lid[c, p, noffs[k] + jj] = n
            pos_in_class += 1
    return class_list, NN, F_pad, row_slots, node_gid, node_lid


def _build_program(class_list, NN, F):
    """Emit the Bass program (same for all cores; per-core data differs)."""
    nc = bacc.Bacc(None, num_swdge_queues=NQUEUES)
    f_1 = nc.declare_dram_parameter("f_1", [N_NODES, 4], dt.float32, isOutput=False)
    pos = nc.declare_dram_parameter("pos", [N_NODES, 3], dt.float32, isOutput=False)
    W1 = nc.declare_dram_parameter("W1", [NUM_BASIS, HIDDEN], dt.float32, isOutput=False)
    W2 = nc.declare_dram_parameter("W2", [HIDDEN, 5], dt.float32, isOutput=False)
    # gather indices (pair ids, wrapped-16) + parity masks
    eidx = nc.declare_dram_parameter("eidx", [P, F * 8], dt.int16, isOutput=False)
    epar = nc.declare_dram_parameter("epar", [P, F], dt.float32, isOutput=False)
    NLPAD = (NN * P + NPG - 1) // NPG * NPG
    gidx = nc.declare_dram_parameter("gidx", [P, NLPAD // 16], dt.int16, isOutput=False)
    gpar = nc.declare_dram_parameter("gpar", [P, NN], dt.float32, isOutput=False)
    sidx = nc.declare_dram_parameter("sidx", [P, NLPAD // 16], dt.int16, isOutput=False)
    yout = nc.declare_dram_parameter("yout", [YROWS, GELEM], dt.float32, isOutput=True)

    rec2 = nc.dram_tensor("rec2", [PAIRS, GELEM], dt.float32)

    n_chunks = F // FC
    C_EMB = 1.14136 * float(np.e) ** 2
    # fold: relu *sqrt(2); W1 /sqrt(10); emb *C ; W2 /sqrt(64); /sqrt(32) nbrs
    w1_scale = C_EMB / math.sqrt(NUM_BASIS)
    w2_common = math.sqrt(2.0) / math.sqrt(HIDDEN) / math.sqrt(32.0)
    col_scales = [
        math.sqrt(0.5) * w2_common,            # a0 (w0 * x0)
        1.0 * w2_common,                       # a1 (w1 * x0 * u)
        (1.0 / math.sqrt(3.0)) * w2_common,    # a2 (w2 * xv)
        math.sqrt(0.5) * w2_common,            # a3 (w3 * dot)
        (1.0 / math.sqrt(2.0)) * w2_common,    # a4 (w4 * cross)
    ]

    with TileContext(nc) as tc:
        with (
            tc.tile_pool(name="persist", bufs=1) as pp,
            tc.tile_pool(name="chunk", bufs=2) as cp,
            tc.tile_pool(name="recp", bufs=2) as rp,
            tc.tile_pool(name="psum", bufs=2, space="PSUM") as psp,
            tc.tile_pool(name="mmp", bufs=2, space="PSUM") as mmp,
        ):
            # ---- stage 0: build pair-packed record table in DRAM ----
            ztile = pp.tile([P, GELEM], dt.float32)
            nc.vector.memset(ztile[:], 0.0)
            nc.sync.dma_start(out=rec2[PAIRS - 24:PAIRS, :], in_=ztile[:24, :])
            G0 = 64
            BLK = P * G0  # 8192 rows per full block
            nfull = N_NODES // BLK
            for b in range(nfull):
                r0 = b * BLK
                t = cp.tile([P, G0, REC], dt.float32, tag="recb", name="recb")
                nc.sync.dma_start(out=t[:, :, 0:3], in_=pos[r0:r0 + BLK, :])
                nc.sync.dma_start(out=t[:, :, 3:7], in_=f_1[r0:r0 + BLK, :])
                nc.sync.dma_start(
                    out=rec2[r0 // 2:(r0 + BLK) // 2, 0:16],
                    in_=t[:].rearrange("p (g two) r -> p g (two r)", two=2))
            # tail in 256-row sub-blocks ([128 partitions x 2 rows])
            r0 = nfull * BLK
            while r0 < N_NODES:
                r1 = min(r0 + 2 * P, N_NODES)
                m = (r1 - r0) // 2  # pair count (N_NODES even; r0 even)
                tt = cp.tile([P, 2, REC], dt.float32, tag="recbt", name="recbt")
                nc.sync.dma_start(out=tt[:m, :, 0:3], in_=pos[r0:r1, :])
                nc.sync.dma_start(out=tt[:m, :, 3:7], in_=f_1[r0:r1, :])
                nc.sync.dma_start(
                    out=rec2[r0 // 2:r1 // 2, 0:16],
                    in_=tt[:m].rearrange("p two r -> p (two r)"))
                r0 = r1

            # ---- stage 0b: weights prep (bf16 for the PE matmuls) ----
            # block-diagonal weights: 2 edge-columns per matmul set
            w1blk = pp.tile([2 * NUM_BASIS, P], dt.float32)
            nc.vector.memset(w1blk[:], 0.0)
            nc.sync.dma_start(out=w1blk[0:NUM_BASIS, 0:HIDDEN], in_=W1[:])
            nc.sync.dma_start(out=w1blk[NUM_BASIS:2 * NUM_BASIS, HIDDEN:2 * HIDDEN],
                              in_=W1[:])
            nc.vector.tensor_scalar_mul(w1blk[:], w1blk[:], w1_scale)
            w1b16 = pp.tile([2 * NUM_BASIS, P], dt.bfloat16)
            nc.vector.tensor_copy(out=w1b16[:], in_=w1blk[:])
            w2blk = pp.tile([P, 2 * 5], dt.float32)
            nc.vector.memset(w2blk[:], 0.0)
            nc.sync.dma_start(out=w2blk[0:HIDDEN, 0:5], in_=W2[:])
            nc.sync.dma_start(out=w2blk[HIDDEN:2 * HIDDEN, 5:10], in_=W2[:])
            for j, s in enumerate(col_scales):
                nc.vector.tensor_scalar_mul(w2blk[:, j:j + 1], w2blk[:, j:j + 1], s)
                nc.vector.tensor_scalar_mul(w2blk[:, 5 + j:6 + j], w2blk[:, 5 + j:6 + j], s)
            w2b16 = pp.tile([P, 2 * 5], dt.bfloat16)
            nc.vector.tensor_copy(out=w2b16[:], in_=w2blk[:])
            ident = pp.tile([P, P], dt.float32)
            kconst = pp.tile([P, 1, NUM_BASIS], dt.float32)
            for k in range(NUM_BASIS):
                nc.vector.memset(kconst[:, :, k:k + 1], k + 1.0)
            from concourse.masks import make_identity
            make_identity(nc, ident[:])

            # ---- stage 1: load index arrays + parity masks ----
            eidxt = pp.tile([P, F * 8], dt.int16)
            nc.sync.dma_start(out=eidxt[:], in_=eidx[:])
            epart = pp.tile([P, F], dt.float32)
            nc.sync.dma_start(out=epart[:], in_=epar[:])
            gidxt = pp.tile([P, NLPAD // 16], dt.int16)
            nc.sync.dma_start(out=gidxt[:], in_=gidx[:])
            gpart = pp.tile([P, NN], dt.float32)
            nc.sync.dma_start(out=gpart[:], in_=gpar[:])
            sidxt = pp.tile([P, NLPAD // 16], dt.int16)
            nc.sync.dma_start(out=sidxt[:], in_=sidx[:])

            # ---- stage 2: gather packed col-node records, select halves ----
            NPCALLS = NLPAD // NPG
            gpc = pp.tile([P, NLPAD // P, 16], dt.float32, name="gpc")
            for j in range(NPCALLS):
                _dma_gather64(
                    nc,
                    gpc[:, j * (NPG // P):(j + 1) * (NPG // P), :],
                    rec2[:, 0:16],
                    gidxt[:, j * (NPG // 16):(j + 1) * (NPG // 16)],
                    NPG, j % NQUEUES)
            pcol = pp.tile([P, NN, REC], dt.float32, name="pcol")
            ga = gpc[:, 0:NN, 0:REC]
            gb = gpc[:, 0:NN, REC:2 * REC]
            nc.vector.tensor_tensor(out=pcol[:], in0=gb, in1=ga, op=Alu.subtract)
            nc.vector.tensor_tensor(
                out=pcol[:], in0=pcol[:],
                in1=gpart[:].rearrange("p (n one) -> p n one", one=1)
                .to_broadcast([P, NN, REC]),
                op=Alu.mult)
            nc.vector.tensor_tensor(out=pcol[:], in0=pcol[:], in1=ga, op=Alu.add)

            # expand pos[col] to slot-aligned slabs [P, F] per component
            pcx = pp.tile([P, F], dt.float32, tag="pcx", name="pcx")
            pcy = pp.tile([P, F], dt.float32, tag="pcy", name="pcy")
            pcz = pp.tile([P, F], dt.float32, tag="pcz", name="pcz")
            foff = 0
            noff = 0
            for (k, nk) in class_list:
                for comp, dst in ((0, pcx), (1, pcy), (2, pcz)):
                    src = pcol[:, noff:noff + nk, comp:comp + 1]  # [P, nk, 1]
                    nc.vector.tensor_copy(
                        out=dst[:, foff:foff + nk * k].rearrange(
                            "p (n d) -> p n d", d=k),
                        in_=src.to_broadcast([P, nk, k]),
                    )
                foff += nk * k
                noff += nk

            # persistent 4-group sums [P, F/4] per component
            F8 = F // 4
            g8 = [pp.tile([P, F8], dt.float32, tag=f"g8_{i}", name=f"g8_{i}") for i in range(4)]

            # ---- stage 3: per-chunk pipeline ----
            gq = NPCALLS  # global gather-call counter for queue round-robin
            for ch in range(n_chunks):
                c0 = ch * FC
                gblk = rp.tile([P, FC, 16], dt.float32, tag="gblk", name="gblk")
                for j in range(FC * P // NPG):
                    _dma_gather64(
                        nc,
                        gblk[:, j * (NPG // P):(j + 1) * (NPG // P), :],
                        rec2[:, 0:16],
                        eidxt[:, ch * FC * 8 + j * (NPG // 16):
                              ch * FC * 8 + (j + 1) * (NPG // 16)],
                        NPG, gq % NQUEUES)
                    gq += 1
                recc = cp.tile([P, FC, REC], dt.float32, tag="recc", name="recc")
                ra = gblk[:, :, 0:REC]
                rb = gblk[:, :, REC:2 * REC]
                mpar = epart[:, c0:c0 + FC].rearrange(
                    "p (f one) -> p f one", one=1).to_broadcast([P, FC, REC])
                nc.vector.tensor_tensor(out=recc[:], in0=rb, in1=ra, op=Alu.subtract)
                nc.vector.tensor_tensor(out=recc[:], in0=recc[:], in1=mpar, op=Alu.mult)
                nc.vector.tensor_tensor(out=recc[:], in0=recc[:], in1=ra, op=Alu.add)

                prx = recc[:, :, 0]
                pry = recc[:, :, 1]
                prz = recc[:, :, 2]
                x0 = recc[:, :, 3]
                x1 = recc[:, :, 4]
                x2 = recc[:, :, 5]
                x3 = recc[:, :, 6]

                def T(tag):
                    return cp.tile([P, FC], dt.float32, tag=tag, name=tag)

                evx, evy, evz = T("evx"), T("evy"), T("evz")
                nc.vector.tensor_tensor(out=evx[:], in0=prx, in1=pcx[:, c0:c0 + FC], op=Alu.subtract)
                nc.vector.tensor_tensor(out=evy[:], in0=pry, in1=pcy[:, c0:c0 + FC], op=Alu.subtract)
                nc.vector.tensor_tensor(out=evz[:], in0=prz, in1=pcz[:, c0:c0 + FC], op=Alu.subtract)
                r2 = T("r2")
                tmp = T("tmp")
                nc.vector.tensor_tensor(out=r2[:], in0=evx[:], in1=evx[:], op=Alu.mult)
                nc.vector.tensor_tensor(out=tmp[:], in0=evy[:], in1=evy[:], op=Alu.mult)
                nc.vector.tensor_tensor(out=r2[:], in0=r2[:], in1=tmp[:], op=Alu.add)
                nc.vector.tensor_tensor(out=tmp[:], in0=evz[:], in1=evz[:], op=Alu.mult)
                nc.vector.tensor_tensor(out=r2[:], in0=r2[:], in1=tmp[:], op=Alu.add)
                nc.vector.tensor_scalar_max(r2[:], r2[:], 1e-12)
                r = T("r")
                nc.scalar.sqrt(out=r[:], in_=r2[:])
                rinv = T("rinv")
                nc.vector.reciprocal_approx_fast(out=rinv[:], in_=r[:])
                ux, uy, uz = T("ux"), T("uy"), T("uz")
                nc.vector.tensor_tensor(out=ux[:], in0=evx[:], in1=rinv[:], op=Alu.mult)
                nc.vector.tensor_tensor(out=uy[:], in0=evy[:], in1=rinv[:], op=Alu.mult)
                nc.vector.tensor_tensor(out=uz[:], in0=evz[:], in1=rinv[:], op=Alu.mult)
                # e3nn (y,z,x) permuted unit vector
                up1, up2, up3 = uy, uz, ux

                # ---- embedding [P, FC, 10], batched over basis ----
                # d_k = s - (k+1) with s = 11r/3; emb = exp(2/(d^2-1)) inside
                # |d|<1.  den = min(d^2-1, -1e-9) makes outside-bin values
                # map to exp(-2e9) = 0 exactly -- no mask needed.
                s = T("s")
                nc.vector.tensor_scalar_mul(s[:], r[:], 11.0 / 3.0)
                embs = cp.tile([P, FC, NUM_BASIS], dt.float32, tag="embs", name="embs")
                d3 = cp.tile([P, FC, NUM_BASIS], dt.float32, tag="d3", name="d3")
                nc.vector.tensor_tensor(
                    out=d3[:],
                    in0=s[:].rearrange("p (f one) -> p f one", one=1)
                    .to_broadcast([P, FC, NUM_BASIS]),
                    in1=kconst[:].to_broadcast([P, FC, NUM_BASIS]),
                    op=Alu.subtract)
                den3 = cp.tile([P, FC, NUM_BASIS], dt.float32, tag="den3", name="den3")
                nc.scalar.activation(out=den3[:], in_=d3[:], func=Act.Square,
                                     bias=0.0, scale=1.0)
                nc.vector.tensor_scalar(out=den3[:], in0=den3[:], scalar1=-1.0,
                                        scalar2=-1e-9, op0=Alu.add, op1=Alu.min)
                nc.vector.reciprocal_approx_fast(out=d3[:], in_=den3[:])
                # clamp 1/den to >= -60 so the Exp table sees a sane range
                # (exp(-120) already underflows f32 to exactly 0)
                nc.vector.tensor_scalar_max(d3[:], d3[:], -60.0)
                nc.scalar.activation(out=embs[:], in_=d3[:], func=Act.Exp,
                                     bias=0.0, scale=2.0)

                # ---- radial MLP on PE (bf16, 16 cols / 2048 edges / group) ----
                # group g: 4 transposes [128,4,10]->[40,128] (f32), one bf16
                # PSUM->SBUF copy, then 2 block-diag matmul pairs: rows 0:20
                # cover slots 4q+{0,1}, rows 20:40 slots 4q+{2,3}.
                wsl = cp.tile([P, FC, 5], dt.float32, tag="wsl", name="wsl")
                for g in range(0, FC, 16):
                    # halves land at partition 0 / 32 (engine SBUF/PSUM
                    # accesses must start 32-aligned); rows 10-31 are unused.
                    ws2 = cp.tile([52, 512], dt.float32, tag="ws2", name="ws2")
                    for half in range(2):
                        m0 = g + 8 * half
                        embT = psp.tile([2 * NUM_BASIS, 512], dt.float32,
                                        space="PSUM", tag="embT", name="embT")
                        for q in range(4):
                            nc.tensor.transpose(
                                out=embT[:, q * P:(q + 1) * P],
                                in_=embs[:, m0 + 2 * q:m0 + 2 * q + 2, :],
                                identity=ident[:])
                        embTs = cp.tile([2 * NUM_BASIS, 512], dt.bfloat16,
                                        tag="embTs", name="embTs")
                        nc.scalar.copy(out=embTs[:], in_=embT[:])
                        hpsum = mmp.tile([P, 512], dt.float32, space="PSUM",
                                         tag="hpsum", name="hpsum")
                        nc.tensor.matmul(out=hpsum[:], lhsT=w1b16[:],
                                         rhs=embTs[:], start=True, stop=True)
                        hs = cp.tile([P, 512], dt.bfloat16, tag="hs", name="hs")
                        nc.vector.tensor_scalar_max(hs[:], hpsum[:], 0.0)
                        wpsum = mmp.tile([2 * 5, 512], dt.float32, space="PSUM",
                                         tag="wpsum", name="wpsum")
                        nc.tensor.matmul(out=wpsum[:], lhsT=w2b16[:], rhs=hs[:],
                                         start=True, stop=True)
                        nc.vector.tensor_copy(
                            out=ws2[half * 32:half * 32 + 10, :], in_=wpsum[:])
                    # batch the w transposes: [52, 128] -> [128, 52] per
                    # 128-col block, 64-col padded output blocks
                    wT2 = psp.tile([P, 4, 64], dt.float32, space="PSUM",
                                   tag="wT", name="wT")
                    for q in range(4):
                        nc.tensor.transpose(
                            out=wT2[:, q, 0:52],
                            in_=ws2[:, q * P:(q + 1) * P],
                            identity=ident[:52, :52])
                    # wT2[p, q, u2*32 + u1*5 + j] = w_j(slot g + 8*u2 + 2q + u1)
                    nc.vector.tensor_copy(
                        out=wsl[:, g:g + 16, :].rearrange(
                            "p (u2 q u1) j -> p q u2 (u1 j)", u2=2, q=4, u1=2),
                        in_=wT2[:].rearrange(
                            "p q (u2 c2) -> p q u2 c2", u2=2, c2=32)[:, :, :, 0:10])

                w0 = wsl[:, :, 0]
                w1_ = wsl[:, :, 1]
                w2_ = wsl[:, :, 2]
                w3 = wsl[:, :, 3]
                w4 = wsl[:, :, 4]

                # ---- tensor product ----
                dv = T("dv")
                nc.vector.tensor_tensor(out=dv[:], in0=x1, in1=up1[:], op=Alu.mult)
                nc.vector.tensor_tensor(out=tmp[:], in0=x2, in1=up2[:], op=Alu.mult)
                nc.vector.tensor_tensor(out=dv[:], in0=dv[:], in1=tmp[:], op=Alu.add)
                nc.vector.tensor_tensor(out=tmp[:], in0=x3, in1=up3[:], op=Alu.mult)
                nc.vector.tensor_tensor(out=dv[:], in0=dv[:], in1=tmp[:], op=Alu.add)

                o0 = T("o0")
                nc.vector.tensor_tensor(out=o0[:], in0=w0, in1=x0, op=Alu.mult)
                nc.vector.tensor_tensor(out=tmp[:], in0=w3, in1=dv[:], op=Alu.mult)
                nc.vector.tensor_tensor(out=o0[:], in0=o0[:], in1=tmp[:], op=Alu.add)

                t1 = T("t1")
                nc.vector.tensor_tensor(out=t1[:], in0=w1_, in1=x0, op=Alu.mult)

                # cross(xv, up)
                cr1, cr2, cr3 = T("cr1"), T("cr2"), T("cr3")
                nc.vector.tensor_tensor(out=cr1[:], in0=x2, in1=up3[:], op=Alu.mult)
                nc.vector.tensor_tensor(out=tmp[:], in0=x3, in1=up2[:], op=Alu.mult)
                nc.vector.tensor_tensor(out=cr1[:], in0=cr1[:], in1=tmp[:], op=Alu.subtract)
                nc.vector.tensor_tensor(out=cr2[:], in0=x3, in1=up1[:], op=Alu.mult)
                nc.vector.tensor_tensor(out=tmp[:], in0=x1, in1=up3[:], op=Alu.mult)
                nc.vector.tensor_tensor(out=cr2[:], in0=cr2[:], in1=tmp[:], op=Alu.subtract)
                nc.vector.tensor_tensor(out=cr3[:], in0=x1, in1=up2[:], op=Alu.mult)
                nc.vector.tensor_tensor(out=tmp[:], in0=x2, in1=up1[:], op=Alu.mult)
                nc.vector.tensor_tensor(out=cr3[:], in0=cr3[:], in1=tmp[:], op=Alu.subtract)

                o1, o2, o3 = T("o1"), T("o2"), T("o3")
                for oo, upc, xc, crc in ((o1, up1, x1, cr1), (o2, up2, x2, cr2),
                                         (o3, up3, x3, cr3)):
                    nc.vector.tensor_tensor(out=oo[:], in0=t1[:], in1=upc[:], op=Alu.mult)
                    nc.vector.tensor_tensor(out=tmp[:], in0=w2_, in1=xc, op=Alu.mult)
                    nc.vector.tensor_tensor(out=oo[:], in0=oo[:], in1=tmp[:], op=Alu.add)
                    nc.vector.tensor_tensor(out=tmp[:], in0=w4, in1=crc[:], op=Alu.mult)
                    nc.vector.tensor_tensor(out=oo[:], in0=oo[:], in1=tmp[:], op=Alu.add)

                # ---- 4-group partial reduction into persistent slabs ----
                for i, oo in enumerate((o0, o1, o2, o3)):
                    nc.vector.tensor_reduce(
                        out=g8[i][:, ch * FC // 4:(ch + 1) * FC // 4],
                        in_=oo[:].rearrange("p (g e) -> p g e", e=4),
                        op=Alu.add,
                        axis=mybir.AxisListType.X,
                    )

            # ---- stage 4: per-class final reduction [P, NN] x4 ----
            # padded to NLPAD//P columns so stage-5 scatter calls never
            # straddle the tile boundary (pad columns scatter zeros to trash
            # rows).
            nsum = pp.tile([P, NLPAD // P, 4], dt.float32, name="nsum")
            nc.vector.memset(nsum[:, NN:, :], 0.0)
            foff8 = 0
            noff = 0
            for (k, nk) in class_list:
                k8 = k // 4
                for i in range(4):
                    nc.vector.tensor_reduce(
                        out=nsum[:, noff:noff + nk, i],
                        in_=g8[i][:, foff8:foff8 + nk * k8].rearrange(
                            "p (n g) -> p n g", g=k8),
                        op=Alu.add,
                        axis=mybir.AxisListType.X,
                    )
                foff8 += nk * k8
                noff += nk

            # ---- stage 5: scatter-add per-node sums into strided yout ----
            # trash rows (lid >= NPC) absorb dummy-node and pad slots.
            for j in range(NLPAD // NPG):
                j0 = j * (NPG // P)
                j1 = (j + 1) * (NPG // P)
                nc.gpsimd.dma_scatter_add(
                    yout[:, 0:4], nsum[:, j0:j1, :],
                    sidxt[:, j * (NPG // 16):(j + 1) * (NPG // 16)],
                    NPG, NPG, 4, elem_step=GELEM)

    nc.finalize()
    return nc


def kernel(f_1, pos, W1, W2, edge_index):
    f_1 = np.ascontiguousarray(f_1, np.float32)
    pos = np.ascontiguousarray(pos, np.float32)
    W1 = np.ascontiguousarray(W1, np.float32)
    W2 = np.ascontiguousarray(W2, np.float32)
    ei = np.asarray(edge_index).astype(np.int64)

    class_list, NN, F, row_slots, node_gid, node_lid = _build_layout(ei)
    nc = _build_program(class_list, NN, F)

    NLPAD = (NN * P + NPG - 1) // NPG * NPG
    TRASH_PAIR = PAIRS - 6  # a zeroed pair row
    in_maps = []
    for c in range(N_CORES):
        rs = row_slots[c].astype(np.int64)           # [P, F]
        eidx = _wrap16((rs // 2).T.ravel())          # [128, F*8]
        epar = (rs & 1).astype(np.float32)           # [P, F]
        gl = (node_gid[c].astype(np.int64) // 2).T.ravel()
        gl = np.concatenate([gl, np.full(NLPAD - gl.size, TRASH_PAIR, np.int64)])
        gidxw = _wrap16(gl)
        gpar = (node_gid[c] & 1).astype(np.float32)  # [P, NN]
        sl = node_lid[c].astype(np.int64).T.ravel()
        sl = np.concatenate([sl, np.full(NLPAD - sl.size, NPC + 8, np.int64)])
        sidxw = _wrap16(sl)
        in_maps.append({
            "f_1": f_1, "pos": pos, "W1": W1, "W2": W2,
            "eidx": eidx, "epar": epar, "gidx": gidxw, "gpar": gpar,
            "sidx": sidxw,
        })
    trace = os.environ.get("KERNEL_TRACE", "0") == "1"
    if trace:
        _install_ntff_shim()
    res = run_bass_kernel_spmd(nc, in_maps, list(range(N_CORES)), trace=trace)
    global LAST_EXEC_NS
    LAST_EXEC_NS = res.exec_time_ns
    out = np.concatenate(
        [res.results[c]["yout"][:NPC, 0:4] for c in range(N_CORES)], axis=0)
    return np.ascontiguousarray(out.astype(np.float32))


if __name__ == "__main__":
    import reference
    inputs = {k: np.asarray(v) for k, v in reference.setup_inputs().items()}
    out = kernel(**inputs)
    print("kernel out", out.shape, out.dtype)


# revision 31
# speedup vs baseline: 2.2743x; 1.3018x over previous
"""TRN2 Bass kernel for nn_EquivariantConv (GNN message passing).

Strategy (8 NeuronCores):
- Edges assigned to core c by destination node block: col in [c*6250, (c+1)*6250).
- Per core, edges laid out col-node-major with degree padded to multiples of 4,
  packed into 128 SBUF partitions (each dest node's slots live in one
  partition, contiguous along the free dim). This makes:
    * pos[col] a zero-stride broadcast access pattern (free),
    * the segment-sum a dense tensor_reduce - no scatter at all.
- Source-row records (pos|f_1 packed to 8 f32) are gathered from a DRAM table
  via gpsimd dma_gather (SWDGE): the table packs TWO records per 256B row
  (gather element granularity), idx = row//2 fits int16, and a 3-op DVE
  parity select picks the right half. 1024 indices per call (desc-ring cap).
- Radial MLP computed exactly on TensorE per edge chunk (transpose -> matmul
  x2 -> transpose back); basis embedding built on DVE+ACT (reciprocal on ACT).
- Per-node sums scattered with dma_scatter_add into a 256B-strided output
  buffer (trash rows absorb dummy slots); host slices the real rows.

Dummy padding edges point at a zeroed table row -> f_1 = 0 -> f_edge = 0
exactly (all tensor-product terms carry an x factor).
"""

import math
import os
import numpy as np

import concourse.bass as bass
import concourse.bacc as bacc
import concourse.mybir as mybir
from concourse.tile import TileContext
from concourse.bass_utils import run_bass_kernel_spmd

dt = mybir.dt


def _patch_tile_drain():
    """This walrus build rejects drains carrying >1 semaphore wait ("Too many
    sync wait commands"). Split the kernel-tail drain's waits onto separate
    SP drain instructions, one wait each."""
    import concourse.tile as tile_mod
    from concourse.vector_clock import ScopedClock

    if getattr(tile_mod.TileContext, "_drain_patched", False):
        return

    def _drain_and_barrier(self, tick_clock, wait_clock):
        nc = self.nc
        probe = nc.sync.drain()
        wait_clock.add_sem_waits(
            probe.ins, ScopedClock({None: tick_clock.global_clock})
        )
        waits = list(probe.ins.sync_info.on_wait) if probe.ins.sync_info else []
        if len(waits) > 1:
            probe.ins.sync_info.on_wait = waits[:1]
            for w in waits[1:]:
                n2 = nc.sync.drain()
                if n2.ins.sync_info is None:
                    n2.ins.sync_info = mybir.SyncInfo(on_wait=[w], on_update=[])
                else:
                    n2.ins.sync_info.on_wait = [w]
        nc.all_engine_barrier()
        popped = nc._tile_sem_poison_stack.pop()
        assert popped is self._sem_poison
        nc.clear_and_free_semaphores(list(self.sems.allocated().values()))
        nc.all_engine_barrier()

    tile_mod.TileContext._drain_and_barrier = _drain_and_barrier
    tile_mod.TileContext._drain_patched = True


def _install_ntff_shim():
    """Optional: enable NTFF profiling under axon (antenv.axon_hooks shim)."""
    import contextlib
    import ctypes
    import sys
    import types

    if "antenv.axon_hooks" in sys.modules:
        return
    so_path = "/opt/axon/libaxon_pjrt.so"
    if not os.path.exists(so_path):
        return
    try:
        lib = ctypes.CDLL(so_path)
        if not hasattr(lib, "axon_start_nrt_profile"):
            return
        lib.axon_start_nrt_profile.argtypes = [
            ctypes.POINTER(ctypes.c_int64), ctypes.c_size_t]
        lib.axon_start_nrt_profile.restype = ctypes.c_int64
        lib.axon_stop_nrt_profile.argtypes = [ctypes.c_char_p]
        lib.axon_stop_nrt_profile.restype = ctypes.c_int64

        @contextlib.contextmanager
        def _profile(output_dir, device_ids):
            import jax
            jax.devices()
            if device_ids:
                ids = (ctypes.c_int64 * len(device_ids))(*device_ids)
                rc = lib.axon_start_nrt_profile(ids, len(device_ids))
            else:
                rc = lib.axon_start_nrt_profile(None, 0)
            if rc != 0:
                raise RuntimeError(f"axon_start_nrt_profile rc={rc}")
            try:
                yield
            finally:
                lib.axon_stop_nrt_profile(output_dir.encode())

        mod = types.ModuleType("antenv.axon_hooks")
        mod.get_axon_ntff_profile_hook = lambda: _profile
        mod.set_axon_ntff_profile_hook = lambda h: None
        import antenv
        antenv.axon_hooks = mod
        sys.modules["antenv.axon_hooks"] = mod
    except Exception:
        pass


_patch_tile_drain()

LAST_EXEC_NS = None
Alu = mybir.AluOpType
Act = mybir.ActivationFunctionType

N_NODES = 50000
N_EDGES = 1600000
NUM_BASIS = 10
HIDDEN = 64
MAX_RADIUS = 3.0
N_CORES = 8
NPC = N_NODES // N_CORES  # dest nodes per core
P = 128

# table rows: N_NODES real + rows N_NODES..N_NODES+47 zeroed (dummy target)
TBL_ROWS = N_NODES + 48
REC = 8            # packed record: [pos_x, pos_y, pos_z, f0, f1, f2, f3, pad]
PAIRS = TBL_ROWS // 2          # 25024 gather elements of 2 records / 256B
GELEM = 64                     # gather element: 64 f32 = 256B
DUMMY_ROW = N_NODES            # zeroed record row

FC = 128          # chunk width (free-dim columns); 128*FC slots per chunk
NPG = 1024        # indices per dma_gather/dma_scatter_add call (ring cap)
NQUEUES = 4       # SWDGE queues; round-robin hides per-queue DMA round-trip
YROWS = NPC + 22  # output rows plus trash rows for dummy slots (lid NPC+6)


def _dma_gather64(nc, out_ap, in_ap, idxs_ap, num_idxs, queue_num):
    """dma_gather of 64B elements (16 f32) on a 256B-strided table.

    bass's dma_gather asserts elem_size_bytes % 256 == 0, but that is a
    transpose-mode restriction; the non-transpose ucode only needs the row
    STRIDE to be a 256B multiple (stride_bytes_256 descriptor field).
    Verified on hardware. in_ap must be tbl[:, 0:16] of a [N, 64] f32 tensor.
    """
    g = nc.gpsimd
    stride_bytes = GELEM * 4
    _in_ap = g.lower_ap_dma(in_ap, for_custom_bir_dma=True)
    _idxs_ap = g.lower_ap(idxs_ap)
    _out_ap = g.lower_ap(out_ap)
    return g.add_instruction(
        mybir.InstDMAGatherAnt(
            name=nc.get_next_instruction_name(),
            ins=[*_in_ap, _idxs_ap, g.lower_val_access(g.to_reg(num_idxs))],
            outs=[_out_ap],
            transpose=False,
            num_idxs=num_idxs,
            elem_size=16,
            stride_bytes_256=stride_bytes // 256,
            gen_mode=0,
            single_packet=True,
            queue_num=queue_num,
            sbuf_tokens_per_rank=0,
            sbuf_free_dim_per_rank=0,
            sbuf_free_dim_pad_per_rank=0,
            sbuf_byte_offset=0,
        ))


def _wrap16(lin):
    """[N] int array -> [128, N/16] int16 wrapped-16, replicated across the 8
    gpsimd cores (partition p holds lin[j*16 + p%16] at free pos j)."""
    n = lin.shape[0]
    assert n % 16 == 0
    w16 = lin.reshape(n // 16, 16).T.astype(np.int16)  # [16, n/16]
    return np.tile(w16, (8, 1))                        # [128, n/16]


def _build_layout(edge_index):
    """Host-side index work: per-core slot layout. Values untouched.

    Cross-core class balancing: per-partition class counts n_k are chosen
    globally from suffix maxima of per-core padded-degree histograms; cores
    short on class-k nodes promote lower-degree nodes into the larger class
    (extra slots become dummy edges). This removes most cross-core padding.
    """
    row = edge_index[0].astype(np.int64)
    col = edge_index[1].astype(np.int64)
    core = col // NPC

    per_core = []
    for c in range(N_CORES):
        m = core == c
        row_c = row[m]
        col_c = col[m] - c * NPC
        deg = np.bincount(col_c, minlength=NPC)
        order = np.argsort(col_c, kind="stable")
        row_sorted = row_c[order]
        starts = np.zeros(NPC + 1, np.int64)
        np.cumsum(deg, out=starts[1:])
        nz = np.nonzero(deg)[0]
        pdeg = ((deg[nz] + 3) // 4) * 4
        per_core.append((deg, starts, row_sorted, nz, pdeg))

    # global class sizing: S_k = max over cores of #nodes with pdeg >= k
    all_k = sorted({int(v) for (_, _, _, _, pdeg) in per_core for v in pdeg},
                   reverse=True)
    n_k = {}
    cum = 0  # per-partition slots already committed to classes > k
    for k in all_k:
        s_k = max(int((pd >= k).sum()) for (_, _, _, _, pd) in per_core)
        need = max((s_k + P - 1) // P, cum)
        n_k[k] = need - cum
        cum = need
    class_list = [(k, n_k[k]) for k in all_k if n_k[k] > 0]
    class_list = class_list[::-1]  # ascending k, as the device program expects

    NN = sum(nk for (_, nk) in class_list)
    F = sum(nk * k for (k, nk) in class_list)
    F_pad = (F + FC - 1) // FC * FC

    row_slots = np.full((N_CORES, P, F_pad), DUMMY_ROW, np.int32)
    node_gid = np.full((N_CORES, P, NN), DUMMY_ROW, np.int32)
    node_lid = np.full((N_CORES, P, NN), NPC + 6, np.int32)

    # per-class slot bases (ascending class order = device layout order)
    foffs = {}
    noffs = {}
    fo = 0
    no = 0
    for (k, nk) in class_list:
        foffs[k] = fo
        noffs[k] = no
        fo += nk * k
        no += nk

    desc = [k for (k, _) in class_list][::-1]
    for c in range(N_CORES):
        deg, starts, row_sorted, nz, pdeg = per_core[c]
        # nodes sorted by padded degree desc; assign to class slots desc
        order = np.argsort(-pdeg, kind="stable")
        nodes_desc = nz[order]
        pos_in_class = 0
        ki = 0
        for n in nodes_desc:
            while pos_in_class >= n_k[desc[ki]] * P:
                ki += 1
                pos_in_class = 0
            k = desc[ki]
            j = pos_in_class  # global slot index within class k
            p = j % P
            jj = j // P
            d = deg[n]
            f0 = foffs[k] + jj * k
            row_slots[c, p, f0:f0 + d] = row_sorted[starts[n]:starts[n + 1]]
            node_gid[c, p, noffs[k] + jj] = c * NPC + n
            node_lid[c, p, noffs[k] + jj] = n
            pos_in_class += 1
    return class_list, NN, F_pad, row_slots, node_gid, node_lid


def _build_program(class_list, NN, F):
    """Emit the Bass program (same for all cores; per-core data differs)."""
    nc = bacc.Bacc(None, num_swdge_queues=NQUEUES)
    f_1 = nc.declare_dram_parameter("f_1", [N_NODES, 4], dt.float32, isOutput=False)
    pos = nc.declare_dram_parameter("pos", [N_NODES, 3], dt.float32, isOutput=False)
    W1 = nc.declare_dram_parameter("W1", [NUM_BASIS, HIDDEN], dt.float32, isOutput=False)
    W2 = nc.declare_dram_parameter("W2", [HIDDEN, 5], dt.float32, isOutput=False)
    # gather indices (pair ids, wrapped-16) + parity masks
    eidx = nc.declare_dram_parameter("eidx", [P, F * 8], dt.int16, isOutput=False)
    epar = nc.declare_dram_parameter("epar", [P, F], dt.float32, isOutput=False)
    NLPAD = (NN * P + NPG - 1) // NPG * NPG
    gidx = nc.declare_dram_parameter("gidx", [P, NLPAD // 16], dt.int16, isOutput=False)
    gpar = nc.declare_dram_parameter("gpar", [P, NN], dt.float32, isOutput=False)
    sidx = nc.declare_dram_parameter("sidx", [P, NLPAD // 16], dt.int16, isOutput=False)
    yout = nc.declare_dram_parameter("yout", [YROWS, GELEM], dt.float32, isOutput=True)

    rec2 = nc.dram_tensor("rec2", [PAIRS, GELEM], dt.float32)

    n_chunks = F // FC
    C_EMB = 1.14136 * float(np.e) ** 2
    # fold: relu *sqrt(2); W1 /sqrt(10); emb *C ; W2 /sqrt(64); /sqrt(32) nbrs
    w1_scale = C_EMB / math.sqrt(NUM_BASIS)
    w2_common = math.sqrt(2.0) / math.sqrt(HIDDEN) / math.sqrt(32.0)
    col_scales = [
        math.sqrt(0.5) * w2_common,            # a0 (w0 * x0)
        1.0 * w2_common,                       # a1 (w1 * x0 * u)
        (1.0 / math.sqrt(3.0)) * w2_common,    # a2 (w2 * xv)
        math.sqrt(0.5) * w2_common,            # a3 (w3 * dot)
        (1.0 / math.sqrt(2.0)) * w2_common,    # a4 (w4 * cross)
    ]

    with TileContext(nc) as tc:
        with (
            tc.tile_pool(name="persist", bufs=1) as pp,
            tc.tile_pool(name="chunk", bufs=2) as cp,
            tc.tile_pool(name="recp", bufs=2) as rp,
            tc.tile_pool(name="psum", bufs=2, space="PSUM") as psp,
            tc.tile_pool(name="mmp", bufs=2, space="PSUM") as mmp,
        ):
            # ---- stage 0: build pair-packed record table in DRAM ----
            ztile = pp.tile([P, GELEM], dt.float32)
            nc.vector.memset(ztile[:], 0.0)
            nc.sync.dma_start(out=rec2[PAIRS - 24:PAIRS, :], in_=ztile[:24, :])
            G0 = 64
            BLK = P * G0  # 8192 rows per full block
            nfull = N_NODES // BLK
            for b in range(nfull):
                r0 = b * BLK
                t = cp.tile([P, G0, REC], dt.float32, tag="recb", name="recb")
                nc.sync.dma_start(out=t[:, :, 0:3], in_=pos[r0:r0 + BLK, :])
                nc.sync.dma_start(out=t[:, :, 3:7], in_=f_1[r0:r0 + BLK, :])
                # bounce through a 256B-row padded tile so the rec2 write is
                # one big contiguous descriptor per partition (the strided
                # 64B-per-row write costs ~48ns/descriptor on the DMA engines)
                t2 = cp.tile([P, G0 // 2, GELEM], dt.float32, tag="recb2",
                             name="recb2")
                nc.vector.tensor_copy(
                    out=t2[:, :, 0:16].rearrange(
                        "p g (two r) -> p g two r", two=2),
                    in_=t[:].rearrange("p (g two) r -> p g two r", two=2))
                nc.sync.dma_start(
                    out=rec2[r0 // 2:(r0 + BLK) // 2, :], in_=t2[:])
            # tail in 256-row sub-blocks ([128 partitions x 2 rows])
            r0 = nfull * BLK
            while r0 < N_NODES:
                r1 = min(r0 + 2 * P, N_NODES)
                m = (r1 - r0) // 2  # pair count (N_NODES even; r0 even)
                tt = cp.tile([P, 2, REC], dt.float32, tag="recbt", name="recbt")
                nc.sync.dma_start(out=tt[:m, :, 0:3], in_=pos[r0:r1, :])
                nc.sync.dma_start(out=tt[:m, :, 3:7], in_=f_1[r0:r1, :])
                nc.sync.dma_start(
                    out=rec2[r0 // 2:r1 // 2, 0:16],
                    in_=tt[:m].rearrange("p two r -> p (two r)"))
                r0 = r1

            # ---- stage 0b: weights prep (bf16 for the PE matmuls) ----
            # block-diagonal weights: 2 edge-columns per matmul set
            w1blk = pp.tile([2 * NUM_BASIS, P], dt.float32)
            nc.vector.memset(w1blk[:], 0.0)
            nc.sync.dma_start(out=w1blk[0:NUM_BASIS, 0:HIDDEN], in_=W1[:])
            nc.sync.dma_start(out=w1blk[NUM_BASIS:2 * NUM_BASIS, HIDDEN:2 * HIDDEN],
                              in_=W1[:])
            nc.vector.tensor_scalar_mul(w1blk[:], w1blk[:], w1_scale)
            w1b16 = pp.tile([2 * NUM_BASIS, P], dt.bfloat16)
            nc.vector.tensor_copy(out=w1b16[:], in_=w1blk[:])
            w2blk = pp.tile([P, 2 * 5], dt.float32)
            nc.vector.memset(w2blk[:], 0.0)
            nc.sync.dma_start(out=w2blk[0:HIDDEN, 0:5], in_=W2[:])
            nc.sync.dma_start(out=w2blk[HIDDEN:2 * HIDDEN, 5:10], in_=W2[:])
            for j, s in enumerate(col_scales):
                nc.vector.tensor_scalar_mul(w2blk[:, j:j + 1], w2blk[:, j:j + 1], s)
                nc.vector.tensor_scalar_mul(w2blk[:, 5 + j:6 + j], w2blk[:, 5 + j:6 + j], s)
            w2b16 = pp.tile([P, 2 * 5], dt.bfloat16)
            nc.vector.tensor_copy(out=w2b16[:], in_=w2blk[:])
            ident = pp.tile([P, P], dt.float32)
            identb = pp.tile([P, P], dt.bfloat16)
            kconst = pp.tile([P, 1, NUM_BASIS], dt.float32)
            for k in range(NUM_BASIS):
                nc.vector.memset(kconst[:, :, k:k + 1], k + 1.0)
            from concourse.masks import make_identity
            make_identity(nc, ident[:])
            nc.vector.tensor_copy(out=identb[:], in_=ident[:])

            # ---- stage 1: load index arrays + parity masks ----
            eidxt = pp.tile([P, F * 8], dt.int16)
            nc.sync.dma_start(out=eidxt[:], in_=eidx[:])
            epart = pp.tile([P, F], dt.float32)
            nc.sync.dma_start(out=epart[:], in_=epar[:])
            gidxt = pp.tile([P, NLPAD // 16], dt.int16)
            nc.sync.dma_start(out=gidxt[:], in_=gidx[:])
            gpart = pp.tile([P, NN], dt.float32)
            nc.sync.dma_start(out=gpart[:], in_=gpar[:])
            sidxt = pp.tile([P, NLPAD // 16], dt.int16)
            nc.sync.dma_start(out=sidxt[:], in_=sidx[:])

            # ---- stage 2: gather packed col-node records, select halves ----
            NPCALLS = NLPAD // NPG
            gpc = pp.tile([P, NLPAD // P, 16], dt.float32, name="gpc")
            for j in range(NPCALLS):
                _dma_gather64(
                    nc,
                    gpc[:, j * (NPG // P):(j + 1) * (NPG // P), :],
                    rec2[:, 0:16],
                    gidxt[:, j * (NPG // 16):(j + 1) * (NPG // 16)],
                    NPG, j % NQUEUES)
            pcol = pp.tile([P, NN, REC], dt.float32, name="pcol")
            ga = gpc[:, 0:NN, 0:REC]
            gb = gpc[:, 0:NN, REC:2 * REC]
            nc.vector.tensor_tensor(out=pcol[:], in0=gb, in1=ga, op=Alu.subtract)
            nc.vector.tensor_tensor(
                out=pcol[:], in0=pcol[:],
                in1=gpart[:].rearrange("p (n one) -> p n one", one=1)
                .to_broadcast([P, NN, REC]),
                op=Alu.mult)
            nc.vector.tensor_tensor(out=pcol[:], in0=pcol[:], in1=ga, op=Alu.add)

            # expand pos[col] to slot-aligned slabs [P, F] per component
            pcx = pp.tile([P, F], dt.float32, tag="pcx", name="pcx")
            pcy = pp.tile([P, F], dt.float32, tag="pcy", name="pcy")
            pcz = pp.tile([P, F], dt.float32, tag="pcz", name="pcz")
            foff = 0
            noff = 0
            for (k, nk) in class_list:
                for comp, dst in ((0, pcx), (1, pcy), (2, pcz)):
                    src = pcol[:, noff:noff + nk, comp:comp + 1]  # [P, nk, 1]
                    nc.vector.tensor_copy(
                        out=dst[:, foff:foff + nk * k].rearrange(
                            "p (n d) -> p n d", d=k),
                        in_=src.to_broadcast([P, nk, k]),
                    )
                foff += nk * k
                noff += nk

            # persistent 4-group sums [P, F/4] per component
            F8 = F // 4
            g8 = [pp.tile([P, F8], dt.float32, tag=f"g8_{i}", name=f"g8_{i}") for i in range(4)]

            # ---- stage 3: per-chunk pipeline ----
            gq = NPCALLS  # global gather-call counter for queue round-robin
            for ch in range(n_chunks):
                c0 = ch * FC
                gblk = rp.tile([P, FC, 16], dt.float32, tag="gblk", name="gblk")
                for j in range(FC * P // NPG):
                    _dma_gather64(
                        nc,
                        gblk[:, j * (NPG // P):(j + 1) * (NPG // P), :],
                        rec2[:, 0:16],
                        eidxt[:, ch * FC * 8 + j * (NPG // 16):
                              ch * FC * 8 + (j + 1) * (NPG // 16)],
                        NPG, gq % NQUEUES)
                    gq += 1
                recc = cp.tile([P, FC, REC], dt.float32, tag="recc", name="recc")
                ra = gblk[:, :, 0:REC]
                rb = gblk[:, :, REC:2 * REC]
                mpar = epart[:, c0:c0 + FC].rearrange(
                    "p (f one) -> p f one", one=1).to_broadcast([P, FC, REC])
                nc.vector.tensor_tensor(out=recc[:], in0=rb, in1=ra, op=Alu.subtract)
                nc.vector.tensor_tensor(out=recc[:], in0=recc[:], in1=mpar, op=Alu.mult)
                nc.vector.tensor_tensor(out=recc[:], in0=recc[:], in1=ra, op=Alu.add)

                prx = recc[:, :, 0]
                pry = recc[:, :, 1]
                prz = recc[:, :, 2]
                x0 = recc[:, :, 3]
                x1 = recc[:, :, 4]
                x2 = recc[:, :, 5]
                x3 = recc[:, :, 6]

                def T(tag):
                    return cp.tile([P, FC], dt.float32, tag=tag, name=tag)

                evx, evy, evz = T("evx"), T("evy"), T("evz")
                nc.vector.tensor_tensor(out=evx[:], in0=prx, in1=pcx[:, c0:c0 + FC], op=Alu.subtract)
                nc.vector.tensor_tensor(out=evy[:], in0=pry, in1=pcy[:, c0:c0 + FC], op=Alu.subtract)
                nc.vector.tensor_tensor(out=evz[:], in0=prz, in1=pcz[:, c0:c0 + FC], op=Alu.subtract)
                r2 = T("r2")
                tmp = T("tmp")
                nc.vector.tensor_tensor(out=r2[:], in0=evx[:], in1=evx[:], op=Alu.mult)
                nc.vector.tensor_tensor(out=tmp[:], in0=evy[:], in1=evy[:], op=Alu.mult)
                nc.vector.tensor_tensor(out=r2[:], in0=r2[:], in1=tmp[:], op=Alu.add)
                nc.vector.tensor_tensor(out=tmp[:], in0=evz[:], in1=evz[:], op=Alu.mult)
                nc.vector.tensor_tensor(out=r2[:], in0=r2[:], in1=tmp[:], op=Alu.add)
                nc.vector.tensor_scalar_max(r2[:], r2[:], 1e-12)
                r = T("r")
                nc.scalar.sqrt(out=r[:], in_=r2[:])
                rinv = T("rinv")
                nc.vector.reciprocal_approx_fast(out=rinv[:], in_=r[:])
                ux, uy, uz = T("ux"), T("uy"), T("uz")
                nc.vector.tensor_tensor(out=ux[:], in0=evx[:], in1=rinv[:], op=Alu.mult)
                nc.vector.tensor_tensor(out=uy[:], in0=evy[:], in1=rinv[:], op=Alu.mult)
                nc.vector.tensor_tensor(out=uz[:], in0=evz[:], in1=rinv[:], op=Alu.mult)
                # e3nn (y,z,x) permuted unit vector
                up1, up2, up3 = uy, uz, ux

                # ---- embedding [P, FC, 10], batched over basis ----
                # d_k = s - (k+1) with s = 11r/3; emb = exp(2/(d^2-1)) inside
                # |d|<1.  den = min(d^2-1, -1e-9) makes outside-bin values
                # map to exp(-2e9) = 0 exactly -- no mask needed.
                s = T("s")
                nc.vector.tensor_scalar_mul(s[:], r[:], 11.0 / 3.0)
                embs = cp.tile([P, FC, NUM_BASIS], dt.bfloat16, tag="embs", name="embs")
                d3 = cp.tile([P, FC, NUM_BASIS], dt.float32, tag="d3", name="d3")
                nc.vector.tensor_tensor(
                    out=d3[:],
                    in0=s[:].rearrange("p (f one) -> p f one", one=1)
                    .to_broadcast([P, FC, NUM_BASIS]),
                    in1=kconst[:].to_broadcast([P, FC, NUM_BASIS]),
                    op=Alu.subtract)
                den3 = cp.tile([P, FC, NUM_BASIS], dt.float32, tag="den3", name="den3")
                nc.scalar.activation(out=den3[:], in_=d3[:], func=Act.Square,
                                     bias=0.0, scale=1.0)
                nc.vector.tensor_scalar(out=den3[:], in0=den3[:], scalar1=-1.0,
                                        scalar2=-1e-9, op0=Alu.add, op1=Alu.min)
                nc.vector.reciprocal_approx_fast(out=d3[:], in_=den3[:])
                # clamp 1/den to >= -60 so the Exp table sees a sane range
                # (exp(-120) already underflows f32 to exactly 0)
                nc.vector.tensor_scalar_max(d3[:], d3[:], -60.0)
                nc.scalar.activation(out=embs[:], in_=d3[:], func=Act.Exp,
                                     bias=0.0, scale=2.0)

                # ---- radial MLP on PE (bf16, 16 cols / 2048 edges / group) ----
                # group g: 4 transposes [128,4,10]->[40,128] (f32), one bf16
                # PSUM->SBUF copy, then 2 block-diag matmul pairs: rows 0:20
                # cover slots 4q+{0,1}, rows 20:40 slots 4q+{2,3}.
                wsl = cp.tile([P, FC, 5], dt.float32, tag="wsl", name="wsl")
                for g in range(0, FC, 16):
                    # halves land at partition 0 / 32 (engine SBUF/PSUM
                    # accesses must start 32-aligned); rows 10-31 are unused.
                    ws2 = cp.tile([42, 512], dt.float32, tag="ws2", name="ws2")
                    wpsum = mmp.tile([42, 512], dt.float32, space="PSUM",
                                     tag="wpsum", name="wpsum")
                    for half in range(2):
                        m0 = g + 8 * half
                        embT = psp.tile([2 * NUM_BASIS, 512], dt.bfloat16,
                                        space="PSUM", tag="embT", name="embT")
                        for q in range(4):
                            nc.tensor.transpose(
                                out=embT[:, q * P:(q + 1) * P],
                                in_=embs[:, m0 + 2 * q:m0 + 2 * q + 2, :],
                                identity=identb[:])
                        embTs = cp.tile([2 * NUM_BASIS, 512], dt.bfloat16,
                                        tag="embTs", name="embTs")
                        nc.scalar.copy(out=embTs[:], in_=embT[:])
                        hpsum = mmp.tile([P, 512], dt.float32, space="PSUM",
                                         tag="hpsum", name="hpsum")
                        nc.tensor.matmul(out=hpsum[:], lhsT=w1b16[:],
                                         rhs=embTs[:], start=True, stop=True)
                        hs = cp.tile([P, 512], dt.bfloat16, tag="hs", name="hs")
                        nc.scalar.activation(out=hs[:], in_=hpsum[:], func=Act.Relu)
                        nc.tensor.matmul(out=wpsum[half * 32:half * 32 + 10, :],
                                         lhsT=w2b16[:], rhs=hs[:],
                                         start=True, stop=True)
                    nc.vector.tensor_copy(out=ws2[:], in_=wpsum[:])
                    # batch the w transposes: [52, 128] -> [128, 52] per
                    # 128-col block, 64-col padded output blocks
                    wT2 = psp.tile([P, 4, 64], dt.float32, space="PSUM",
                                   tag="wT", name="wT")
                    for q in range(4):
                        nc.tensor.transpose(
                            out=wT2[:, q, 0:42],
                            in_=ws2[:, q * P:(q + 1) * P],
                            identity=ident[:42, :42])
                    # wT2[p, q, u2*32 + u1*5 + j] = w_j(slot g + 8*u2 + 2q + u1)
                    nc.vector.tensor_copy(
                        out=wsl[:, g:g + 16, :].rearrange(
                            "p (u2 q u1) j -> p q u2 (u1 j)", u2=2, q=4, u1=2),
                        in_=wT2[:].rearrange(
                            "p q (u2 c2) -> p q u2 c2", u2=2, c2=32)[:, :, :, 0:10])

                w0 = wsl[:, :, 0]
                w1_ = wsl[:, :, 1]
                w2_ = wsl[:, :, 2]
                w3 = wsl[:, :, 3]
                w4 = wsl[:, :, 4]

                # ---- tensor product ----
                dv = T("dv")
                nc.vector.tensor_tensor(out=dv[:], in0=x1, in1=up1[:], op=Alu.mult)
                nc.vector.tensor_tensor(out=tmp[:], in0=x2, in1=up2[:], op=Alu.mult)
                nc.vector.tensor_tensor(out=dv[:], in0=dv[:], in1=tmp[:], op=Alu.add)
                nc.vector.tensor_tensor(out=tmp[:], in0=x3, in1=up3[:], op=Alu.mult)
                nc.vector.tensor_tensor(out=dv[:], in0=dv[:], in1=tmp[:], op=Alu.add)

                o0 = T("o0")
                nc.vector.tensor_tensor(out=o0[:], in0=w0, in1=x0, op=Alu.mult)
                nc.vector.tensor_tensor(out=tmp[:], in0=w3, in1=dv[:], op=Alu.mult)
                nc.vector.tensor_tensor(out=o0[:], in0=o0[:], in1=tmp[:], op=Alu.add)

                t1 = T("t1")
                nc.vector.tensor_tensor(out=t1[:], in0=w1_, in1=x0, op=Alu.mult)

                # cross(xv, up)
                cr1, cr2, cr3 = T("cr1"), T("cr2"), T("cr3")
                nc.vector.tensor_tensor(out=cr1[:], in0=x2, in1=up3[:], op=Alu.mult)
                nc.vector.tensor_tensor(out=tmp[:], in0=x3, in1=up2[:], op=Alu.mult)
                nc.vector.tensor_tensor(out=cr1[:], in0=cr1[:], in1=tmp[:], op=Alu.subtract)
                nc.vector.tensor_tensor(out=cr2[:], in0=x3, in1=up1[:], op=Alu.mult)
                nc.vector.tensor_tensor(out=tmp[:], in0=x1, in1=up3[:], op=Alu.mult)
                nc.vector.tensor_tensor(out=cr2[:], in0=cr2[:], in1=tmp[:], op=Alu.subtract)
                nc.vector.tensor_tensor(out=cr3[:], in0=x1, in1=up2[:], op=Alu.mult)
                nc.vector.tensor_tensor(out=tmp[:], in0=x2, in1=up1[:], op=Alu.mult)
                nc.vector.tensor_tensor(out=cr3[:], in0=cr3[:], in1=tmp[:], op=Alu.subtract)

                o1, o2, o3 = T("o1"), T("o2"), T("o3")
                for oo, upc, xc, crc in ((o1, up1, x1, cr1), (o2, up2, x2, cr2),
                                         (o3, up3, x3, cr3)):
                    nc.vector.tensor_tensor(out=oo[:], in0=t1[:], in1=upc[:], op=Alu.mult)
                    nc.vector.tensor_tensor(out=tmp[:], in0=w2_, in1=xc, op=Alu.mult)
                    nc.vector.tensor_tensor(out=oo[:], in0=oo[:], in1=tmp[:], op=Alu.add)
                    nc.vector.tensor_tensor(out=tmp[:], in0=w4, in1=crc[:], op=Alu.mult)
                    nc.vector.tensor_tensor(out=oo[:], in0=oo[:], in1=tmp[:], op=Alu.add)

                # ---- 4-group partial reduction into persistent slabs ----
                for i, oo in enumerate((o0, o1, o2, o3)):
                    nc.vector.tensor_reduce(
                        out=g8[i][:, ch * FC // 4:(ch + 1) * FC // 4],
                        in_=oo[:].rearrange("p (g e) -> p g e", e=4),
                        op=Alu.add,
                        axis=mybir.AxisListType.X,
                    )

            # ---- stage 4: per-class final reduction [P, NN] x4 ----
            # padded to NLPAD//P columns so stage-5 scatter calls never
            # straddle the tile boundary (pad columns scatter zeros to trash
            # rows).
            nsum = pp.tile([P, NLPAD // P, 4], dt.float32, name="nsum")
            nc.vector.memset(nsum[:, NN:, :], 0.0)
            foff8 = 0
            noff = 0
            for (k, nk) in class_list:
                k8 = k // 4
                for i in range(4):
                    nc.vector.tensor_reduce(
                        out=nsum[:, noff:noff + nk, i],
                        in_=g8[i][:, foff8:foff8 + nk * k8].rearrange(
                            "p (n g) -> p n g", g=k8),
                        op=Alu.add,
                        axis=mybir.AxisListType.X,
                    )
                foff8 += nk * k8
                noff += nk

            # ---- stage 5: scatter-add per-node sums into strided yout ----
            # trash rows (lid >= NPC) absorb dummy-node and pad slots.
            for j in range(NLPAD // NPG):
                j0 = j * (NPG // P)
                j1 = (j + 1) * (NPG // P)
                nc.gpsimd.dma_scatter_add(
                    yout[:, 0:4], nsum[:, j0:j1, :],
                    sidxt[:, j * (NPG // 16):(j + 1) * (NPG // 16)],
                    NPG, NPG, 4, elem_step=GELEM)

    nc.finalize()
    return nc


def kernel(f_1, pos, W1, W2, edge_index):
    f_1 = np.ascontiguousarray(f_1, np.float32)
    pos = np.ascontiguousarray(pos, np.float32)
    W1 = np.ascontiguousarray(W1, np.float32)
    W2 = np.ascontiguousarray(W2, np.float32)
    ei = np.asarray(edge_index).astype(np.int64)

    class_list, NN, F, row_slots, node_gid, node_lid = _build_layout(ei)
    nc = _build_program(class_list, NN, F)

    NLPAD = (NN * P + NPG - 1) // NPG * NPG
    TRASH_PAIR = PAIRS - 6  # a zeroed pair row
    in_maps = []
    for c in range(N_CORES):
        rs = row_slots[c].astype(np.int64)           # [P, F]
        eidx = _wrap16((rs // 2).T.ravel())          # [128, F*8]
        epar = (rs & 1).astype(np.float32)           # [P, F]
        gl = (node_gid[c].astype(np.int64) // 2).T.ravel()
        gl = np.concatenate([gl, np.full(NLPAD - gl.size, TRASH_PAIR, np.int64)])
        gidxw = _wrap16(gl)
        gpar = (node_gid[c] & 1).astype(np.float32)  # [P, NN]
        sl = node_lid[c].astype(np.int64).T.ravel()
        sl = np.concatenate([sl, np.full(NLPAD - sl.size, NPC + 8, np.int64)])
        sidxw = _wrap16(sl)
        in_maps.append({
            "f_1": f_1, "pos": pos, "W1": W1, "W2": W2,
            "eidx": eidx, "epar": epar, "gidx": gidxw, "gpar": gpar,
            "sidx": sidxw,
        })
    trace = os.environ.get("KERNEL_TRACE", "0") == "1"
    if trace:
        _install_ntff_shim()
    res = run_bass_kernel_spmd(nc, in_maps, list(range(N_CORES)), trace=trace)
    global LAST_EXEC_NS
    LAST_EXEC_NS = res.exec_time_ns
    out = np.concatenate(
        [res.results[c]["yout"][:NPC, 0:4] for c in range(N_CORES)], axis=0)
    return np.ascontiguousarray(out.astype(np.float32))


if __name__ == "__main__":
    import reference
    inputs = {k: np.asarray(v) for k, v in reference.setup_inputs().items()}
    out = kernel(**inputs)
    print("kernel out", out.shape, out.dtype)
